# revision 1
# baseline (speedup 1.0000x reference)
"""DeltaNet fused kernel for 8 trn2 NeuronCores.

Sharding: core c handles (b = c//4, h = c%4).  Within each 4-core group
(same batch), heads are tensor-parallel; cross-head reductions (gate
stats, output projection) go through small AllGather/AllReduce
collectives.  Host pre-transposes hidden_states to (D, L) so every
matmul contracts over the partition axis natively.

Layout conventions on device (per core):
  - "feat-major": channels on SBUF partitions, tokens on the free axis.
    Projections, FIR convs, B-twin matmuls, stats reductions.
  - "token-major": tokens on partitions.  Scan outputs, inv-apply.
  - chunk C=128 delta rule; (I-A)^-1 via commuting-factor repeated
    squaring with transposed-twin maintenance (no PE transposes there).
"""
import math
import numpy as np

import concourse.bass as bass
import concourse.bacc as bacc
import concourse.mybir as mybir
from concourse import tile
from concourse.bass_utils import run_bass_kernel_spmd

F32 = mybir.dt.float32
AF = mybir.ActivationFunctionType
ALU = mybir.AluOpType

# ---------------- static problem config ----------------
B, L, D, H = 2, 4096, 1024, 4
DK = DV = D // H          # 256
C = 128                   # delta chunk
NCHUNK = L // C           # 32
LT = 512                  # projection/streaming token tile
NLT = L // LT             # 8
KT = D // 128             # 8 contraction tiles
GROUPS = [[0, 1, 2, 3], [4, 5, 6, 7]]
EPS_RMS = 1e-5

# engine split for fir_long noise taps (63 total, tap 63 = identity):
N_LONG_PE = 44            # taps 0..43 on PE (diag matmul, PSUM accum)
N_LONG_GP = 6             # taps 44..49 on GPSIMD
# remaining 50..62 on DVE, identity tap 63 fused into combine

# Wcat column layout (feat-major projection output rows)
#   q: 0..255, k: 256..511, v: 512..767, g1: 768..1023,
#   og1: 1024..1151, scal: 1152..1159  [beta_pre, res_pre, 0...]
NM = 10                   # M-tiles of 128 (last holds 8 scal rows padded)
MCOLS = NM * 128          # 1280 padded columns

def _np_f32(x):
    return np.ascontiguousarray(x, dtype=np.float32)


def build_host_inputs(inputs):
    """Returns per-core input maps (list of 8 dicts)."""
    hs = _np_f32(inputs['hidden_states'])
    Wq, Wk, Wv = (_np_f32(inputs[k]) for k in ('Wq', 'Wk', 'Wv'))
    Wb, Wres = _np_f32(inputs['Wb']), _np_f32(inputs['Wres'])
    Wg1, Wog1 = _np_f32(inputs['Wg1']), _np_f32(inputs['Wog1'])
    Wg2, Wog2 = _np_f32(inputs['Wg2']), _np_f32(inputs['Wog2'])
    bg2 = _np_f32(inputs['bg2'])
    wq_c, wk_c, wv_c = (_np_f32(inputs[k]) for k in ('wq_conv', 'wk_conv', 'wv_conv'))
    fir_long = _np_f32(inputs['fir_long']).reshape(D, 64)
    fir_short = _np_f32(inputs['fir_short']).reshape(D, 5)
    Wo = _np_f32(inputs['Wo'])
    logit_temp = float(np.asarray(inputs['logit_temp'])[0])
    conv_res_logit = _np_f32(inputs['conv_res_logit'])
    norm_w = _np_f32(inputs['norm_w'])
    bg1 = _np_f32(inputs['bg1']); bog1 = _np_f32(inputs['bog1'])
    bres = _np_f32(inputs['bres'])
    bog2 = float(np.asarray(inputs['bog2'])[0])

    hsT = [np.ascontiguousarray(hs[b].T) for b in range(B)]  # (D, L)

    # strict-upper mask and identity
    idx = np.arange(C)
    su = (idx[:, None] < idx[None, :]).astype(np.float32)
    ident = np.eye(C, dtype=np.float32)
    ident8 = np.zeros((8, 8), np.float32); np.fill_diagonal(ident8, 1.0)

    maps = []
    for core in range(8):
        b, h = core // 4, core % 4
        sl = slice(h * 256, (h + 1) * 256)
        og_sl = slice(h * 128, (h + 1) * 128)
        wcat = np.zeros((D, MCOLS), np.float32)
        wcat[:, 0:256] = Wq[:, sl]
        wcat[:, 256:512] = Wk[:, sl]
        wcat[:, 512:768] = Wv[:, sl]
        wcat[:, 768:1024] = Wg1[:D, sl]
        wcat[:, 1024:1152] = Wog1[:D, og_sl]
        wcat[:, 1152] = Wb[:, h]
        wcat[:, 1153] = Wres[:, h]
        # conv filters for this head's channels (256, K) -> per ch-tile (128,)
        wq_h, wk_h, wv_h = wq_c[sl], wk_c[sl], wv_c[sl]
        fl_h, fs_h = fir_long[sl], fir_short[sl]
        # PE diag tap matrices for fir_long taps 0..N_LONG_PE-1, both ch tiles
        # lhsT layout: (128, 128) diag(w) per (tap, tile)
        diag_taps = np.zeros((N_LONG_PE, 2, 128, 128), np.float32)
        for t in range(N_LONG_PE):
            for ft in range(2):
                np.fill_diagonal(diag_taps[t, ft], fl_h[ft * 128:(ft + 1) * 128, t])
        # gate-weight tails (stats part) as lhsT: Wg1[1024:1040] (16, slice256)
        wg1_st = np.ascontiguousarray(Wg1[D:D + 16, sl])        # (16, 256)
        wog1_st = np.ascontiguousarray(Wog1[D:D + 8, og_sl])    # (8, 128)
        # broadcast selector matrices (see device row layout)
        sel44 = np.zeros((4, 4 * 128), np.float32)
        for j in range(4):
            sel44[j, j * 128:(j + 1) * 128] = 1.0
        selrs = np.zeros((4, 4), np.float32); selrs[3, 0] = 1.0
        sel8 = np.zeros((8, 4 * 128), np.float32)
        for j in range(4):
            sel8[4 + j, j * 128:(j + 1) * 128] = 1.0
        colsel = np.zeros((128, 32 * 32), np.float32)
        for r in range(32):
            colsel[:, 32 * r + r] = 1.0
        # second-layer gate weights: Wg2 slice rows for this head (256, 4)
        wg2_h = np.concatenate([Wg2[sl][:128], Wg2[sl][128:]], axis=1)  # (128, 8)
        wog2_h = np.ascontiguousarray(Wog2[og_sl, :])           # (128, 1)
        # output projection lhsT: rows = full D contraction after AG.
        # After AG, o_full^T is (1024, L) with head j's blocked rows.
        # Core outputs columns slice of out: Wo[:, out_sl] (1024, 256)
        wo_cols = np.ascontiguousarray(Wo[:, sl])               # (1024, 256)
        # biases / misc per-head
        bg1_h = np.ascontiguousarray(bg1[sl])                   # (256,)
        bog1_h = np.ascontiguousarray(bog1[og_sl])              # (128,)
        inv_temp = 1.0 / math.log1p(math.exp(logit_temp))
        static_h = 1.0 / (1.0 + math.exp(-conv_res_logit[h]))
        maps.append({
            'hsT': hsT[b],
            'wcat': wcat,
            'diag_taps': diag_taps.reshape(N_LONG_PE * 2 * 128, 128),
            'convw': np.concatenate([    # per-partition scalar columns (256, n)
                wq_h, wk_h, wv_h, fl_h, fs_h], axis=1),  # 4+4+4+64+5 = 81
            'su': su, 'ident': ident, 'ident8': ident8,
            'wg1_st': wg1_st, 'wog1_st': wog1_st,
            'wg2_h': wg2_h, 'wog2_h': wog2_h,
            'wo_cols': wo_cols,
            'bg1_h': bg1_h.reshape(1, -1), 'bog1_h': bog1_h.reshape(1, -1),
            'bg2': (bg2 * inv_temp).reshape(1, 4),
            'invt4': np.full((4, 1), inv_temp, np.float32),
            'bias2': np.array([[0.0], [bres[h]]], np.float32),
            'consts': np.array([inv_temp, static_h, bres[h], bog2,
                                norm_w[0]], np.float32).reshape(1, 5),
            'norm_w': norm_w.reshape(1, DV),
            'sres_col': np.array([[1.0], [static_h]], np.float32),
            'sel44': sel44, 'selrs': selrs, 'sel8': sel8, 'colsel': colsel,
        })
    return maps


# ---------------- device program ----------------
# Row layout in the shared "rows" SBUF tile.  Matmul-operand blocks must
# start at partition 0/32/64; broadcasts use host-built selector matrices.
R_RQ, R_RK, R_BETA, R_RESS = 0, 1, 2, 3    # block A (base 0), bcast via sel44
R_RAW = 8                                   # 8..27: [ls,ll,do,vd] x 5 raw rows
R_SCAL0 = 32                                # block B (base 32): 8 scal rows
R_ORAW = 40                                 # 40..44: o-branch raw rows (DMA only)
R_M0 = 64                                   # block C (base 64): m0..m3
R_P0 = 68                                   # p0..p3 (DMA-written)
NROWS = 72

def build_program():
    nc = bacc.Bacc("TRN2", target_bir_lowering=False, debug=True)
    dt = F32
    dram = {}
    def din(name, shape):
        dram[name] = nc.dram_tensor(name, list(shape), dt, kind="ExternalInput")
        return dram[name]
    def dout(name, shape):
        dram[name] = nc.dram_tensor(name, list(shape), dt, kind="ExternalOutput")
        return dram[name]
    def dint(name, shape, shared=False):
        kw = {"addr_space": "Shared"} if shared else {}
        dram[name] = nc.dram_tensor(name, list(shape), dt, **kw)
        return dram[name]

    hsT = din('hsT', (D, L))
    wcat = din('wcat', (D, MCOLS))
    diag_taps = din('diag_taps', (N_LONG_PE * 2 * 128, 128))
    convw = din('convw', (256, 81))
    su_d = din('su', (C, C)); ident_d = din('ident', (C, C))
    ident8_d = din('ident8', (8, 8))
    wg1_st = din('wg1_st', (16, 256)); wog1_st = din('wog1_st', (8, 128))
    wg2_h = din('wg2_h', (128, 8)); wog2_h = din('wog2_h', (128, 1))
    wo_cols = din('wo_cols', (D, 256))
    bg1_h = din('bg1_h', (1, 256)); bog1_h = din('bog1_h', (1, 128))
    bg2_d = din('bg2', (1, 4))
    invt4_d = din('invt4', (4, 1))
    bias2_d = din('bias2', (2, 1))
    consts_d = din('consts', (1, 5))
    sres_d = din('sres_col', (2, 1))
    norm_w_d = din('norm_w', (1, DV))
    msel_d = din('msel', (80, 16))
    sel4_d = din('sel44', (4, 512)); selrs_d = din('selrs', (4, 4))
    sel8_d = din('sel8', (8, 512)); colsel_d = din('colsel', (128, 1024))
    msel2_d = din('msel2', (20, 4))

    outT = dout('outT', (256, L))

    v_sp = dint('v_sp', (2, 128, L))
    q_sp = dint('q_sp', (2, 128, L))
    k_sp = dint('k_sp', (2, 128, L))
    o_sp = dint('o_sp', (2, 128, L))
    ll_sp = dint('ll_sp', (2, 128, L))
    ls_sp = dint('ls_sp', (2, 128, L))
    gh_sp = dint('gh_sp', (2, 128, L))
    ogh_sp = dint('ogh_sp', (128, L))
    onrm_sp = dint('onrm_sp', (2, 128, L))
    doT_sp = dint('doT_sp', (2, 128, L))
    ag1_in = dint('ag1_in', (20, L)); ag1_out = dint('ag1_out', (80, L))
    lg_in = dint('lg_in', (4, L));   lg_out = dint('lg_out', (4, L))
    ag2_in = dint('ag2_in', (5, L)); ag2_out = dint('ag2_out', (20, L))
    ar3_in = dint('ar3_in', (1, L)); ar3_out = dint('ar3_out', (1, L))
    ag4_out = dint('ag4_out', (4, 2, 128, L))

    with tile.TileContext(nc) as tc:
        _build(nc, tc, dram)
        if os.environ.get("KERN_SPILL_OUT"):
            for nm in ('v_sp', 'q_sp', 'k_sp', 'o_sp', 'll_sp', 'ls_sp',
                       'gh_sp', 'ogh_sp', 'onrm_sp', 'doT_sp'):
                t = dram[nm]
                dbg = nc.dram_tensor(nm + '_dbg', list(t.shape), F32,
                                     kind="ExternalOutput")
                dram[nm + '_dbg'] = dbg
                nc.sync.dma_start(dbg[:], t[:])
    nc.compile()
    return nc


def _build(nc, tc, dram):
    from contextlib import ExitStack
    ctx = ExitStack()
    with ctx:
        _build_inner(nc, tc, dram, ctx)


import os
PHASES = int(os.environ.get("KERN_PHASES", "3"))
NCHUNK_DBG = int(os.environ.get("KERN_NCHUNK", str(NCHUNK)))
DSTEP = int(os.environ.get("KERN_DSTEP", "9"))


def _build_inner(nc, tc, dram, ctx):
    """Emit the full per-core program inside one TileContext.

    Pools are phase-scoped (stack allocator): projection-phase pools exit
    before the delta phase opens, so SBUF/PSUM peak is per-phase."""
    MM = nc.tensor.matmul
    V, A, G = nc.vector, nc.scalar, nc.gpsimd
    X = mybir.AxisListType.X

    # ---- persistent pools ----
    P_const = ctx.enter_context(tc.tile_pool(name="const", bufs=1))
    P_big = ctx.enter_context(tc.tile_pool(name="big", bufs=1))

    su_sb = P_const.tile([C, C], F32, tag="su")
    nc.sync.dma_start(su_sb[:], dram['su'][:])
    ident_sb = P_const.tile([C, C], F32, tag="ident")
    nc.sync.dma_start(ident_sb[:], dram['ident'][:])
    ident8_t = P_const.tile([40, 8], F32, tag="ident8t")
    nc.sync.dma_start(ident8_t[32:40, :], dram['ident8'][:])
    slm_sb = P_const.tile([C, C], F32, tag="slm")
    uim_sb = P_const.tile([C, C], F32, tag="uim")
    V.tensor_tensor(uim_sb[:], su_sb[:], ident_sb[:], ALU.add)
    cw = [P_const.tile([128, 81], F32, tag=f"cw{ft}", name=f"cw{ft}") for ft in range(2)]
    for ft in range(2):
        nc.sync.dma_start(cw[ft][:], dram['convw'][ft * 128:(ft + 1) * 128, :])
    ones_col = P_const.tile([128, 1], F32, tag="ones_col")
    G.memset(ones_col[:], 1.0)
    ones_row = P_const.tile([1, 128], F32, tag="ones_row")
    G.memset(ones_row[:], 1.0)
    ones4_t = P_const.tile([68, 1], F32, tag="ones4t")
    G.memset(ones4_t[:], 1.0)
    o4row = P_const.tile([1, 4], F32, tag="o4row")
    G.memset(o4row[:], 0.02)
    invdv = P_const.tile([128, 128], F32, tag="invdv")
    G.memset(invdv[:], 1.0 / DV)
    eps6 = P_const.tile([128, 1], F32, tag="eps6")
    G.memset(eps6[:], 1e-6)
    eps5 = P_const.tile([128, 1], F32, tag="eps5")
    G.memset(eps5[:], EPS_RMS)
    consts = P_const.tile([1, 5], F32, tag="consts")
    nc.sync.dma_start(consts[:], dram['consts'][:])
    sres_col = P_const.tile([2, 1], F32, tag="srescol")
    nc.sync.dma_start(sres_col[:], dram['sres_col'][:])
    nw_col = P_const.tile([128, 2], F32, tag="nwcol")
    nc.sync.dma_start(nw_col[:], dram['norm_w'][:].rearrange("o (f p) -> (o p) f", p=128))
    bg1c = P_const.tile([128, 2], F32, tag="bg1c")
    nc.sync.dma_start(bg1c[:], dram['bg1_h'][:].rearrange("o (f p) -> (o p) f", p=128))
    bog1c = P_const.tile([128, 1], F32, tag="bog1c")
    nc.sync.dma_start(bog1c[:], dram['bog1_h'][:].rearrange("o (f p) -> (o p) f", p=128))
    bg2c = P_const.tile([4, 1], F32, tag="bg2c")
    nc.sync.dma_start(bg2c[:], dram['bg2'][:].rearrange("o f -> f o"))
    invt4 = P_const.tile([4, 1], F32, tag="invt4")
    nc.sync.dma_start(invt4[:], dram['invt4'][:])
    bias2 = P_const.tile([2, 1], F32, tag="bias2")
    nc.sync.dma_start(bias2[:], dram['bias2'][:])
    wg1_st = P_const.tile([16, 256], F32, tag="wg1st")
    nc.sync.dma_start(wg1_st[:], dram['wg1_st'][:])
    wog1_st = P_const.tile([8, 128], F32, tag="wog1st")
    nc.sync.dma_start(wog1_st[:], dram['wog1_st'][:])
    wg2 = P_const.tile([128, 8], F32, tag="wg2")
    nc.sync.dma_start(wg2[:], dram['wg2_h'][:])
    wog2 = P_const.tile([128, 1], F32, tag="wog2")
    nc.sync.dma_start(wog2[:], dram['wog2_h'][:])
    msel = P_const.tile([80, 16], F32, tag="msel")
    nc.sync.dma_start(msel[:], dram['msel'][:])
    msel2 = P_const.tile([20, 4], F32, tag="msel2")
    nc.sync.dma_start(msel2[:], dram['msel2'][:])
    sel44 = P_const.tile([4, 512], F32, tag="sel44")
    nc.sync.dma_start(sel44[:], dram['sel44'][:])
    selrs = P_const.tile([4, 4], F32, tag="selrs")
    nc.sync.dma_start(selrs[:], dram['selrs'][:])
    colsel = P_const.tile([128, 1024], F32, tag="colsel")
    nc.sync.dma_start(colsel[:], dram['colsel'][:])
    sel8_t = P_const.tile([72, 512], F32, tag="sel8t")
    G.memset(sel8_t[:], 0.0)
    nc.sync.dma_start(sel8_t[64:72, :], dram['sel8'][:])

    rows = P_big.tile([NROWS, L], F32, tag="rows")
    S_t = [P_big.tile([128, DV], F32, tag=f"S{ft}", name=f"S{ft}") for ft in range(2)]

    def raw_r(branch, j):
        return R_RAW + 5 * branch + j
    BR_LS, BR_LL, BR_DO, BR_VD = 0, 1, 2, 3

    # =============== PHASE 1: projections + convs (scoped pools) ========
    with tc.tile_pool(name="wcat", bufs=1) as P_w, \
         tc.tile_pool(name="work", bufs=2) as P_work, \
         tc.tile_pool(name="dtap", bufs=6) as P_dtap, \
         tc.tile_pool(name="ps1", bufs=3, space="PSUM") as P_ps, \
         tc.tile_pool(name="ps1c", bufs=1, space="PSUM") as P_psc, \
         tc.tile_pool(name="ps1r", bufs=2, space="PSUM") as P_psr:

        ps_t = P_psr.tile([C, C], F32, tag="strow")
        nc.tensor.transpose(ps_t[:], su_sb[:], ident_sb[:])
        V.tensor_copy(slm_sb[:], ps_t[:])

        wct = []
        for k in range(KT):
            t = P_w.tile([128, MCOLS], F32, tag=f"wcat{k}")
            nc.sync.dma_start(t[:], dram['wcat'][k * 128:(k + 1) * 128, :])
            wct.append(t)
        raws = {}
        for nm in ("q", "k", "v"):
            for ft in range(2):
                t = P_w.tile([128, 3 + LT], F32, tag=f"raw_{nm}{ft}")
                G.memset(t[:, 0:3], 0.0)
                raws[(nm, ft)] = t
        vroll = []
        for ft in range(2):
            t = P_w.tile([128, 63 + LT], F32, tag=f"vroll{ft}")
            G.memset(t[:, 0:63], 0.0)
            vroll.append(t)

        CW_Q, CW_K, CW_V, CW_FL, CW_FS = 0, 4, 8, 12, 76
        for lt in range(NLT):
            tsl = slice(lt * LT, (lt + 1) * LT)
            hst = []
            for k in range(KT):
                t = P_work.tile([128, LT], F32, tag="hst", bufs=8)
                nc.sync.dma_start(t[:], dram['hsT'][k * 128:(k + 1) * 128, tsl])
                hst.append(t)
            for mt in range(NM):
                ps = P_ps.tile([128, LT], F32, tag="projps")
                for k in range(KT):
                    MM(ps[:], wct[k][:, mt * 128:(mt + 1) * 128], hst[k][:],
                       start=(k == 0), stop=(k == KT - 1))
                if mt < 6:
                    nm, ft = ("q", "k", "v")[mt // 2], mt % 2
                    buf = raws[(nm, ft)]
                    if lt > 0:
                        V.tensor_copy(buf[:, 0:3], buf[:, LT:LT + 3])
                    V.tensor_copy(buf[:, 3:3 + LT], ps[:])
                elif mt < 8:
                    t = P_work.tile([128, LT], F32, tag="ghsb")
                    V.tensor_copy(t[:], ps[:])
                    nc.sync.dma_start(dram['gh_sp'][mt - 6, :, tsl], t[:])
                elif mt == 8:
                    t = P_work.tile([128, LT], F32, tag="ghsb")
                    V.tensor_copy(t[:], ps[:])
                    nc.sync.dma_start(dram['ogh_sp'][:, tsl], t[:])
                else:
                    A.activation(rows[R_SCAL0:R_SCAL0 + 8, tsl], ps[0:8, :], AF.Copy)
            # q/k/v 4-tap conv + SiLU (+ spill q,k,v; sumsq rows for q,k)
            qk_tiles = {}
            for im, nm in enumerate(("q", "k", "v")):
                wof = (CW_Q, CW_K, CW_V)[im]
                eng = V
                for ft in range(2):
                    buf = raws[(nm, ft)]
                    acc = P_work.tile([128, LT], F32, tag="cacc", bufs=3)
                    eng.tensor_scalar(acc[:], buf[:, 0:LT], cw[ft][:, wof:wof + 1],
                                      None, ALU.mult)
                    for t_ in range(1, 4):
                        eng.scalar_tensor_tensor(
                            acc[:], buf[:, t_:t_ + LT],
                            cw[ft][:, wof + t_:wof + t_ + 1],
                            acc[:], ALU.mult, ALU.add)
                    if nm == "v":
                        A.activation(vroll[ft][:, 63:63 + LT], acc[:], AF.Silu)
                        nc.sync.dma_start(dram['v_sp'][ft, :, tsl],
                                          vroll[ft][:, 63:63 + LT])
                    else:
                        t = P_work.tile([128, LT], F32, tag="qkin", bufs=4)
                        A.activation(t[:], acc[:], AF.Silu)
                        nc.sync.dma_start(dram[f'{nm}_sp'][ft, :, tsl], t[:])
                        qk_tiles[(nm, ft)] = t
            # stat-row production psum block: 32 rows via colsel placement
            prod_ps = P_psr.tile([32, LT], F32, tag="strow", name="prod_ps")
            prod_n = [0]
            def prod(ridx, x_ap, first=False, last=False):
                MM(prod_ps[:], colsel[:, 32 * ridx:32 * (ridx + 1)], x_ap,
                   start=first, stop=last)
            # sumsq rows for rq/rk (rows 0,1)
            for ridx, nm in ((R_RQ, "q"), (R_RK, "k")):
                for ft in range(2):
                    x = qk_tiles[(nm, ft)]
                    sq = P_work.tile([128, LT], F32, tag="sqsc", bufs=2)
                    G.tensor_tensor(sq[:], x[:], x[:], ALU.mult)
                    prod(ridx, sq[:], first=(ridx == R_RQ and ft == 0))
            # fir_long + fir_short
            ll_t, ls_t = [None, None], [None, None]
            for ft in range(2):
                psll = P_psc.tile([128, LT], F32, tag=f"llps{ft}")
                for t_ in range(N_LONG_PE):
                    dtp = P_dtap.tile([128, 128], F32, tag="dtap")
                    nc.sync.dma_start(
                        dtp[:],
                        dram['diag_taps'][(t_ * 2 + ft) * 128:(t_ * 2 + ft + 1) * 128, :])
                    MM(psll[:], dtp[:], vroll[ft][:, t_:t_ + LT],
                       start=(t_ == 0), stop=(t_ == N_LONG_PE - 1))
                ch = P_work.tile([128, LT], F32, tag="llch")
                A.activation(ch[:], vroll[ft][:, 63:63 + LT], AF.Copy,
                             scale=cw[ft][:, CW_FL + 63:CW_FL + 64])
                for t_ in range(N_LONG_PE, N_LONG_PE + N_LONG_GP):
                    V.scalar_tensor_tensor(ch[:], vroll[ft][:, t_:t_ + LT],
                                           cw[ft][:, CW_FL + t_:CW_FL + t_ + 1],
                                           ch[:], ALU.mult, ALU.add)
                for t_ in range(N_LONG_PE + N_LONG_GP, 63):
                    V.scalar_tensor_tensor(ch[:], vroll[ft][:, t_:t_ + LT],
                                           cw[ft][:, CW_FL + t_:CW_FL + t_ + 1],
                                           ch[:], ALU.mult, ALU.add)
                ll = P_work.tile([128, LT], F32, tag=f"ll{ft}")
                V.tensor_tensor(ll[:], ch[:], psll[:], ALU.add)
                nc.sync.dma_start(dram['ll_sp'][ft, :, tsl], ll[:])
                ll_t[ft] = ll
                lsb = P_work.tile([128, LT], F32, tag=f"ls{ft}")
                A.activation(lsb[:], vroll[ft][:, 63:63 + LT], AF.Copy,
                             scale=cw[ft][:, CW_FS + 4:CW_FS + 5])
                for t_ in range(4):
                    V.scalar_tensor_tensor(lsb[:], vroll[ft][:, 59 + t_:59 + t_ + LT],
                                           cw[ft][:, CW_FS + t_:CW_FS + t_ + 1],
                                           lsb[:], ALU.mult, ALU.add)
                nc.sync.dma_start(dram['ls_sp'][ft, :, tsl], lsb[:])
                ls_t[ft] = lsb
            # branch raw stat rows (sum, sumsq, abs) for ls, ll, vd
            n_last = 0
            for br, srcs in ((BR_LS, ls_t), (BR_LL, ll_t), (BR_VD, None)):
                for ft in range(2):
                    x = vroll[ft][:, 63:63 + LT] if srcs is None else srcs[ft][:]
                    sq = P_work.tile([128, LT], F32, tag="sqsc", bufs=2)
                    G.tensor_tensor(sq[:], x, x, ALU.mult)
                    ab = P_work.tile([128, LT], F32, tag="absc", bufs=2)
                    A.activation(ab[:], x, AF.Abs)
                    last_branch = (br == BR_VD and ft == 1)
                    prod(raw_r(br, 0), x)
                    prod(raw_r(br, 1), sq[:])
                    prod(raw_r(br, 2), ab[:], last=last_branch)
            V.tensor_copy(rows[0:32, tsl], prod_ps[:])
            if lt < NLT - 1:
                for ft in range(2):
                    V.tensor_copy(vroll[ft][:, 0:63], vroll[ft][:, LT:LT + 63])

        # ---- row fixups: base-0 scratch ops + DMA placement ----
        A.activation(rows[0:2, :], rows[0:2, :], AF.Abs_reciprocal_sqrt,
                     bias=eps6[0:2, 0:1])
        for nt in range(NLT):
            tsl = slice(nt * LT, (nt + 1) * LT)
            bsc = P_work.tile([2, LT], F32, tag="bsc", name="bsc")
            A.activation(bsc[:], rows[R_SCAL0:R_SCAL0 + 2, tsl], AF.Sigmoid,
                         bias=bias2[:, 0:1])
            V.tensor_scalar(bsc[:], bsc[:], sres_col[:, 0:1], None, ALU.mult)
            nc.sync.dma_start(rows[2:4, tsl], bsc[:])
            for br in (BR_LS, BR_LL, BR_VD):
                fsc = P_work.tile([1, LT], F32, tag="fsc", name="fsc")
                nc.sync.dma_start(fsc[:], rows[raw_r(br, 1):raw_r(br, 1) + 1, tsl])
                A.activation(fsc[:], fsc[:], AF.Sqrt)
                nc.sync.dma_start(rows[raw_r(br, 3):raw_r(br, 3) + 1, tsl], fsc[:])
                fsc2 = P_work.tile([1, LT], F32, tag="fsc2", name="fsc2")
                nc.sync.dma_start(fsc2[:], rows[raw_r(br, 0):raw_r(br, 0) + 1, tsl])
                V.tensor_tensor(fsc2[:], fsc2[:], fsc2[:], ALU.mult)
                nc.sync.dma_start(rows[raw_r(br, 4):raw_r(br, 4) + 1, tsl], fsc2[:])

    # =============== PHASE 2: delta rule (chunk C=128) ==================
    if PHASES < 2:
        t0 = P_big.tile([128, LT], F32, tag="dummy", name="t0")
        G.memset(t0[:], 0.0)
        for mt in range(2):
            for nt in range(NLT):
                nc.sync.dma_start(
                    dram['outT'][mt * 128:(mt + 1) * 128,
                                 nt * LT:(nt + 1) * LT], t0[:])
        return
    with tc.tile_pool(name="dl", bufs=2) as P_dl, \
         tc.tile_pool(name="dn", bufs=4, space="PSUM") as P_dn, \
         tc.tile_pool(name="dw", bufs=3, space="PSUM") as P_dw:
        for ft in range(2):
            G.memset(S_t[ft][:], 0.0)

        def bcast(ridx, csl):
            assert 0 <= ridx < 4
            ps = P_dn.tile([128, C], F32, tag="dn", name="bc")
            MM(ps[:], sel44[:, ridx * 128:(ridx + 1) * 128], rows[0:4, csl],
               start=True, stop=True)
            return ps

        for cc in range(NCHUNK_DBG):
            csl = slice(cc * C, (cc + 1) * C)
            qc, kc, vc = [], [], []
            for ft in range(2):
                t = P_dl.tile([128, C], F32, tag=f"qc{ft}", name="qc")
                nc.sync.dma_start(t[:], dram['q_sp'][ft, :, csl]); qc.append(t)
                t = P_dl.tile([128, C], F32, tag=f"kc{ft}", name="kc")
                nc.sync.dma_start(t[:], dram['k_sp'][ft, :, csl]); kc.append(t)
                t = P_dl.tile([128, C], F32, tag=f"vc{ft}", name="vc")
                nc.sync.dma_start(t[:], dram['v_sp'][ft, :, csl]); vc.append(t)
            qn, kn, kb = [], [], []
            psb = bcast(R_RQ, csl)
            for ft in range(2):
                t = P_dl.tile([128, C], F32, tag=f"qn{ft}", name="qn")
                V.tensor_tensor(t[:], qc[ft][:], psb[:], ALU.mult); qn.append(t)
            psb = bcast(R_RK, csl)
            for ft in range(2):
                t = P_dl.tile([128, C], F32, tag=f"kn{ft}", name="kn")
                V.tensor_tensor(t[:], kc[ft][:], psb[:], ALU.mult); kn.append(t)
            psb = bcast(R_BETA, csl)
            for ft in range(2):
                t = P_dl.tile([128, C], F32, tag=f"kb{ft}", name="kb")
                V.tensor_tensor(t[:], kn[ft][:], psb[:], ALU.mult); kb.append(t)
            if DSTEP < 2:
                continue
            ps8 = P_dn.tile([128, 8], F32, tag="dn", name="ps8")
            MM(ps8[:], rows[R_SCAL0:R_SCAL0 + 8, csl], ident8_t[32:40, :],
               start=True, stop=True)
            bcol = P_dl.tile([128, 1], F32, tag="bcol", name="bcol")
            A.activation(bcol[:], ps8[:, 0:1], AF.Sigmoid)
            if DSTEP < 3:
                continue
            vb_tok = P_dl.tile([128, DV], F32, tag="vbtok", name="vbtok")
            kn_tok = P_dl.tile([128, DV], F32, tag="kntok", name="kntok")
            for ft in range(2):
                pst = P_dn.tile([128, C], F32, tag="dn", name="ptr")
                nc.tensor.transpose(pst[:], vc[ft][:], ident_sb[:])
                V.tensor_scalar(vb_tok[:, ft * C:(ft + 1) * C], pst[:],
                                bcol[:, 0:1], None, ALU.mult)
                pst2 = P_dn.tile([128, C], F32, tag="dn", name="ptr2")
                nc.tensor.transpose(pst2[:], kn[ft][:], ident_sb[:])
                V.tensor_copy(kn_tok[:, ft * C:(ft + 1) * C], pst2[:])
            kb_tok = P_dl.tile([128, DV], F32, tag="kbtok", name="kbtok")
            V.tensor_scalar(kb_tok[:], kn_tok[:], bcol[:, 0:1], None, ALU.mult)
            if DSTEP < 4:
                continue
            psB = P_dn.tile([128, C], F32, tag="dn", name="psB")
            for ft in range(2):
                MM(psB[:], kn[ft][:], kb[ft][:], start=(ft == 0), stop=(ft == 1))
            B_t = P_dl.tile([128, C], F32, tag="B0", name="B_t")
            V.scalar_tensor_tensor(B_t[:], psB[:], -1.0, su_sb[:], ALU.mult, ALU.mult)
            psBt = P_dn.tile([128, C], F32, tag="dn", name="psBt")
            for ft in range(2):
                MM(psBt[:], kb[ft][:], kn[ft][:], start=(ft == 0), stop=(ft == 1))
            Bt_t = P_dl.tile([128, C], F32, tag="B1", name="Bt_t")
            V.scalar_tensor_tensor(Bt_t[:], psBt[:], -1.0, slm_sb[:], ALU.mult, ALU.mult)
            if DSTEP < 5:
                continue
            P_m = P_dl.tile([128, C], F32, tag="Pm0", name="P_m")
            V.tensor_tensor(P_m[:], ident_sb[:], B_t[:], ALU.add)
            cur, curT = B_t, Bt_t
            for lvl in range(1, 7):
                if lvl < 6:
                    ps1 = P_dn.tile([128, C], F32, tag="dn", name="sq1")
                    MM(ps1[:], curT[:], cur[:], start=True, stop=True)
                    nxt = P_dl.tile([128, C], F32, tag=f"nx{lvl % 2}", name="nxt")
                    V.tensor_copy(nxt[:], ps1[:])
                else:
                    nxt = None
                ps2 = P_dn.tile([128, C], F32, tag="dn", name="sq2")
                MM(ps2[:], cur[:], curT[:], start=True, stop=True)
                nxtT = P_dl.tile([128, C], F32, tag=f"nxT{lvl % 2}", name="nxtT")
                V.tensor_copy(nxtT[:], ps2[:])
                ps3 = P_dn.tile([128, C], F32, tag="dn", name="sq3")
                MM(ps3[:], nxtT[:], P_m[:], start=True, stop=True)
                Pn = P_dl.tile([128, C], F32, tag=f"Pm{1 + (lvl % 2)}", name="Pn")
                V.tensor_tensor(Pn[:], P_m[:], ps3[:], ALU.add)
                P_m, cur, curT = Pn, nxt, nxtT
            if DSTEP < 6:
                continue
            psu = P_dw.tile([128, DV], F32, tag="dw", name="psu")
            MM(psu[:], P_m[:], vb_tok[:], start=True, stop=True)
            u_tok = P_dl.tile([128, DV], F32, tag="utok", name="u_tok")
            V.tensor_copy(u_tok[:], psu[:])
            wT = []
            for ft in range(2):
                psw = P_dn.tile([128, C], F32, tag="dn", name="psw")
                MM(psw[:], kb_tok[:, ft * C:(ft + 1) * C], P_m[:],
                   start=True, stop=True)
                t = P_dl.tile([128, C], F32, tag=f"wT{ft}", name="wT")
                V.tensor_copy(t[:], psw[:]); wT.append(t)
            if DSTEP < 7:
                continue
            pswS = P_dw.tile([128, DV], F32, tag="dw", name="pswS")
            for ft in range(2):
                MM(pswS[:], wT[ft][:], S_t[ft][:], start=(ft == 0), stop=(ft == 1))
            ui2 = P_dl.tile([128, DV], F32, tag="ui2", name="ui2")
            V.tensor_tensor(ui2[:], u_tok[:], pswS[:], ALU.subtract)
            psA = P_dn.tile([128, C], F32, tag="dn", name="psA")
            for ft in range(2):
                MM(psA[:], kn[ft][:], qn[ft][:], start=(ft == 0), stop=(ft == 1))
            attnT = P_dl.tile([128, C], F32, tag="attnT", name="attnT")
            V.tensor_tensor(attnT[:], psA[:], uim_sb[:], ALU.mult)
            psO = P_dw.tile([128, DV], F32, tag="dw", name="psO")
            MM(psO[:], qn[0][:], S_t[0][:], start=True, stop=False)
            MM(psO[:], qn[1][:], S_t[1][:], start=False, stop=False)
            MM(psO[:], attnT[:], ui2[:], start=False, stop=True)
            do_tok = P_dl.tile([128, DV], F32, tag="dotok", name="do_tok")
            V.tensor_copy(do_tok[:], psO[:])
            for ft in range(2):
                ps = P_dw.tile([128, DV], F32, tag="dw", name="psdS")
                MM(ps[:], kn_tok[:, ft * C:(ft + 1) * C], ui2[:],
                   start=True, stop=True)
                V.tensor_tensor(S_t[ft][:], S_t[ft][:], ps[:], ALU.add)
            if DSTEP < 8:
                continue
            # do raw stats (token-major) -> scatter-add into rows block
            stc = P_dl.tile([128, 32], F32, tag="stc", name="stc")
            G.memset(stc[:], 0.0)
            r0 = raw_r(BR_DO, 0)
            V.tensor_reduce(stc[:, r0:r0 + 1], do_tok[:], X, ALU.add)
            sq_sc = P_dl.tile([128, DV], F32, tag="sqdo", name="sq_sc")
            G.tensor_tensor(sq_sc[:], do_tok[:], do_tok[:], ALU.mult)
            V.tensor_reduce(stc[:, r0 + 1:r0 + 2], sq_sc[:], X, ALU.add)
            V.tensor_reduce(stc[:, r0 + 2:r0 + 3], do_tok[:], X, ALU.add,
                            apply_absolute_value=True)
            psst = P_dn.tile([32, C], F32, tag="dn", name="psst")
            MM(psst[:], stc[:], ident_sb[:], start=True, stop=True)
            V.tensor_tensor(rows[0:32, csl], rows[0:32, csl], psst[:], ALU.add)
            if DSTEP < 9:
                continue
            for ft in range(2):
                pst = P_dn.tile([128, C], F32, tag="dn", name="ptr3")
                nc.tensor.transpose(pst[:], do_tok[:, ft * C:(ft + 1) * C], ident_sb[:])
                t = P_dl.tile([128, C], F32, tag="doT", name="doTt")
                V.tensor_copy(t[:], pst[:])
                nc.sync.dma_start(dram['doT_sp'][ft, :, csl], t[:])
        for nt in range(NLT):
            tsl = slice(nt * LT, (nt + 1) * LT)
            fsc2 = P_dl.tile([1, LT], F32, tag="fsc2", name="fsc2")
            nc.sync.dma_start(fsc2[:], rows[raw_r(BR_DO, 1):raw_r(BR_DO, 1) + 1, tsl])
            A.activation(fsc2[:], fsc2[:], AF.Sqrt)
            nc.sync.dma_start(rows[raw_r(BR_DO, 3):raw_r(BR_DO, 3) + 1, tsl], fsc2[:])
            fsc3 = P_dl.tile([1, LT], F32, tag="fsc3", name="fsc3")
            nc.sync.dma_start(fsc3[:], rows[raw_r(BR_DO, 0):raw_r(BR_DO, 0) + 1, tsl])
            V.tensor_tensor(fsc3[:], fsc3[:], fsc3[:], ALU.mult)
            nc.sync.dma_start(rows[raw_r(BR_DO, 4):raw_r(BR_DO, 4) + 1, tsl], fsc3[:])

    # =============== PHASE 3: gates + combination + output ==============
    if PHASES < 3:
        t1 = P_big.tile([128, LT], F32, tag="dummy", name="t1")
        G.memset(t1[:], 0.0)
        for mt in range(2):
            for nt in range(NLT):
                nc.sync.dma_start(
                    dram['outT'][mt * 128:(mt + 1) * 128,
                                 nt * LT:(nt + 1) * LT], t1[:])
        return
    ones14 = P_const.tile([1, 4], F32, tag="ones14")
    G.memset(ones14[:], 1.0)
    wo_t = []
    for k in range(KT):
        t = P_const.tile([128, 256], F32, tag=f"wo{k}")
        nc.sync.dma_start(t[:], dram['wo_cols'][k * 128:(k + 1) * 128, :])
        wo_t.append(t)

    with tc.tile_pool(name="g", bufs=2) as P_g, \
         tc.tile_pool(name="gb", bufs=4, space="PSUM") as P_gb, \
         tc.tile_pool(name="gps", bufs=2, space="PSUM") as P_gps, \
         tc.tile_pool(name="gsm", bufs=1, space="PSUM") as P_gsm, \
         tc.tile_pool(name="gsr", bufs=1, space="PSUM") as P_gsr:

        stats16 = P_g.tile([16, L], F32, tag="stats16", bufs=1, name="stats16")
        sc2 = P_g.tile([128, L], F32, tag="sc2", bufs=1, name="sc2")
        trow = sc2[0:1, :]
        ogsig = sc2[32:33, :]
        ogp = sc2[64:65, :]
        onesT = P_const.tile([64, 128], F32, tag="onesT", name="onesT")
        G.memset(onesT[:], 1.0)
        # ---- AG1: branch raw stat rows ----
        nc.sync.dma_start(dram['ag1_in'][:], rows[R_RAW:R_RAW + 20, :])
        G.collective_compute("AllGather", ALU.bypass, replica_groups=GROUPS,
                             ins=[dram['ag1_in'][:]], outs=[dram['ag1_out'][:]])
        # stats16 = msel.T @ ag1_out
        for nt in range(NLT):
            tsl = slice(nt * LT, (nt + 1) * LT)
            agt = P_g.tile([80, LT], F32, tag="ag80", name="agt")
            nc.sync.dma_start(agt[:], dram['ag1_out'][:, tsl])
            ps16 = P_gsm.tile([16, LT], F32, tag="gsm", name="ps16")
            MM(ps16[:], msel[:], agt[:], start=True, stop=True)
            V.tensor_copy(stats16[:, tsl], ps16[:])
        # ghid + partial logits
        for nt in range(NLT):
            tsl = slice(nt * LT, (nt + 1) * LT)
            psl = P_gsm.tile([4, LT], F32, tag="gsm", name="psl")
            for mt in range(2):
                psg = P_gps.tile([128, LT], F32, tag="gps", name="psg")
                MM(psg[:], wg1_st[:, mt * 128:(mt + 1) * 128], stats16[:, tsl],
                   start=True, stop=True)
                ghs = P_g.tile([128, LT], F32, tag="ghs", name="ghs")
                nc.sync.dma_start(ghs[:], dram['gh_sp'][mt, :, tsl])
                pre = P_g.tile([128, LT], F32, tag="pre", name="pre")
                V.tensor_tensor(pre[:], ghs[:], psg[:], ALU.add)
                gha = P_g.tile([128, LT], F32, tag="gha", name="gha")
                A.activation(gha[:], pre[:], AF.Gelu, bias=bg1c[:, mt:mt + 1])
                MM(psl[:], wg2[:, mt * 4:(mt + 1) * 4], gha[:],
                   start=(mt == 0), stop=(mt == 1))
            lgt = P_g.tile([4, LT], F32, tag="lgt", name="lgt")
            V.tensor_copy(lgt[:], psl[:])
            nc.sync.dma_start(dram['lg_in'][:, tsl], lgt[:])
        G.collective_compute("AllReduce", ALU.add, replica_groups=GROUPS,
                             ins=[dram['lg_in'][:]], outs=[dram['lg_out'][:]])
        nc.sync.dma_start(rows[R_M0:R_M0 + 4, :], dram['lg_out'][:])
        # e = exp(logits*inv_temp + bg2*inv_temp) ; clipped softmax numerators
        A.activation(rows[R_M0:R_M0 + 4, :], rows[R_M0:R_M0 + 4, :], AF.Exp,
                     bias=bg2c[:, 0:1], scale=invt4[:, 0:1])
        for nt in range(NLT):
            tsl = slice(nt * LT, (nt + 1) * LT)
            pst = P_gsr.tile([1, LT], F32, tag="gsr", name="pst")
            MM(pst[:], ones4_t[64:68, :], rows[R_M0:R_M0 + 4, tsl],
               start=True, stop=True)
            V.tensor_copy(trow[:, tsl], pst[:])
            ps02 = P_gsm.tile([4, LT], F32, tag="gsm", name="ps02")
            MM(ps02[:], o4row[:], trow[:, tsl], start=True, stop=True)
            V.tensor_tensor(rows[R_M0:R_M0 + 4, tsl], rows[R_M0:R_M0 + 4, tsl],
                            ps02[:], ALU.max)
            pst2 = P_gsr.tile([1, LT], F32, tag="gsr", name="pst2")
            MM(pst2[:], ones4_t[64:68, :], rows[R_M0:R_M0 + 4, tsl],
               start=True, stop=True)
            V.tensor_copy(trow[:, tsl], pst2[:])
        A.activation(trow[:], trow[:], AF.Ln)
        A.activation(trow[:], trow[:], AF.Exp, scale=-1.0)
        for nt in range(NLT):
            tsl = slice(nt * LT, (nt + 1) * LT)
            ps4 = P_gsm.tile([4, LT], F32, tag="gsm", name="ps4")
            MM(ps4[:], ones14[:], trow[:, tsl], start=True, stop=True)
            psct = P_g.tile([4, LT], F32, tag="psct", name="psct")
            V.tensor_tensor(psct[:], rows[R_M0:R_M0 + 4, tsl], ps4[:], ALU.mult)
            psrs = P_gsm.tile([4, LT], F32, tag="gsm", name="psrs")
            MM(psrs[:], selrs[:], rows[0:4, tsl], start=True, stop=True)
            V.tensor_tensor(psct[:], psct[:], psrs[:], ALU.add)
            nc.sync.dma_start(rows[R_P0:R_P0 + 4, tsl], psct[:])
        # ---- combination (feat-major) + o raw stats ----
        for nt in range(NLT):
            tsl = slice(nt * LT, (nt + 1) * LT)
            pb = []
            for j in range(4):
                ps = P_gb.tile([128, LT], F32, tag="gb", name="pb")
                MM(ps[:], sel8_t[64:72, j * 128:(j + 1) * 128],
                   rows[R_M0:R_M0 + 8, tsl], start=True, stop=True)
                sb = P_g.tile([128, LT], F32, tag="pbs", bufs=4, name="sb")
                V.tensor_copy(sb[:], ps[:])
                pb.append(sb)
            otiles = []
            for ft in range(2):
                bls = P_g.tile([128, LT], F32, tag="bls", name="bls")
                nc.sync.dma_start(bls[:], dram['ls_sp'][ft, :, tsl])
                bll = P_g.tile([128, LT], F32, tag="bll", name="bll")
                nc.sync.dma_start(bll[:], dram['ll_sp'][ft, :, tsl])
                bdo = P_g.tile([128, LT], F32, tag="bdo", name="bdo")
                nc.sync.dma_start(bdo[:], dram['doT_sp'][ft, :, tsl])
                bvd = P_g.tile([128, LT], F32, tag="bvd", name="bvd")
                nc.sync.dma_start(bvd[:], dram['v_sp'][ft, :, tsl])
                o_t = P_g.tile([128, LT], F32, tag="obr", name="o_t")
                V.tensor_tensor(o_t[:], bls[:], pb[0][:], ALU.mult)
                for br_t, j in ((bll, 1), (bdo, 2), (bvd, 3)):
                    tmp = P_g.tile([128, LT], F32, tag="ctmp", bufs=3, name="tmp")
                    G.tensor_tensor(tmp[:], br_t[:], pb[j][:], ALU.mult)
                    V.tensor_tensor(o_t[:], o_t[:], tmp[:], ALU.add)
                nc.sync.dma_start(dram['o_sp'][ft, :, tsl], o_t[:])
                otiles.append(o_t)
            trip = []
            for ft in range(2):
                sq = P_g.tile([128, LT], F32, tag="osq", bufs=2, name="sq")
                G.tensor_tensor(sq[:], otiles[ft][:], otiles[ft][:], ALU.mult)
                ab = P_g.tile([128, LT], F32, tag="oab", bufs=2, name="ab")
                A.activation(ab[:], otiles[ft][:], AF.Abs)
                trip.append((otiles[ft][:], sq[:], ab[:]))
            for j in range(3):
                psr = P_gsr.tile([1, LT], F32, tag="gsr", name="psr")
                for ft in range(2):
                    MM(psr[:], ones_col[:], trip[ft][j],
                       start=(ft == 0), stop=(ft == 1))
                orsc = P_g.tile([1, LT], F32, tag="orsc", bufs=3, name="orsc")
                V.tensor_copy(orsc[:], psr[:])
                nc.sync.dma_start(rows[R_ORAW + j:R_ORAW + j + 1, tsl], orsc[:])
        for nt in range(NLT):
            tsl = slice(nt * LT, (nt + 1) * LT)
            f1 = P_g.tile([1, LT], F32, tag="orsc", bufs=3, name="f1")
            nc.sync.dma_start(f1[:], rows[R_ORAW + 1:R_ORAW + 2, tsl])
            A.activation(f1[:], f1[:], AF.Sqrt)
            nc.sync.dma_start(rows[R_ORAW + 3:R_ORAW + 4, tsl], f1[:])
            f2 = P_g.tile([1, LT], F32, tag="orsc", bufs=3, name="f2")
            nc.sync.dma_start(f2[:], rows[R_ORAW + 0:R_ORAW + 1, tsl])
            V.tensor_tensor(f2[:], f2[:], f2[:], ALU.mult)
            nc.sync.dma_start(rows[R_ORAW + 4:R_ORAW + 5, tsl], f2[:])
        # ---- AG2 (o stats) -> og hidden -> partial og logit -> AR3 ----
        nc.sync.dma_start(dram['ag2_in'][:], rows[R_ORAW:R_ORAW + 5, :])
        G.collective_compute("AllGather", ALU.bypass, replica_groups=GROUPS,
                             ins=[dram['ag2_in'][:]], outs=[dram['ag2_out'][:]])
        for nt in range(NLT):
            tsl = slice(nt * LT, (nt + 1) * LT)
            ag2t = P_g.tile([20, LT], F32, tag="ag20", name="ag2t")
            nc.sync.dma_start(ag2t[:], dram['ag2_out'][:, tsl])
            ps4o = P_gsm.tile([4, LT], F32, tag="gsm", name="ps4o")
            MM(ps4o[:], msel2[:], ag2t[:], start=True, stop=True)
            og8 = P_g.tile([8, LT], F32, tag="og8", name="og8")
            V.tensor_copy(og8[0:4, :], ps4o[:])
            nc.sync.dma_start(og8[4:8, :], stats16[12:16, tsl])
            psg = P_gps.tile([128, LT], F32, tag="gps", name="psg2")
            MM(psg[:], wog1_st[:], og8[:], start=True, stop=True)
            oghs = P_g.tile([128, LT], F32, tag="ghs", name="oghs")
            nc.sync.dma_start(oghs[:], dram['ogh_sp'][:, tsl])
            pre2 = P_g.tile([128, LT], F32, tag="pre", name="pre2")
            V.tensor_tensor(pre2[:], oghs[:], psg[:], ALU.add)
            ogha = P_g.tile([128, LT], F32, tag="gha", name="ogha")
            A.activation(ogha[:], pre2[:], AF.Gelu, bias=bog1c[:, 0:1])
            psol = P_gsr.tile([1, LT], F32, tag="gsr", name="psol")
            MM(psol[:], wog2[:], ogha[:], start=True, stop=True)
            V.tensor_copy(ogp[:, tsl], psol[:])
        nc.sync.dma_start(dram['ar3_in'][:], ogp[:, :])
        G.collective_compute("AllReduce", ALU.add, replica_groups=GROUPS,
                             ins=[dram['ar3_in'][:]], outs=[dram['ar3_out'][:]])
        nc.sync.dma_start(ogsig[:, :], dram['ar3_out'][:])
        A.activation(ogsig[:, :], ogsig[:, :], AF.Sigmoid, bias=consts[0:1, 3:4])
        # ---- og apply + rmsnorm -> onrm spill ----
        for nt in range(NLT):
            tsl = slice(nt * LT, (nt + 1) * LT)
            ogb = P_gb.tile([128, LT], F32, tag="gb", name="ogb")
            MM(ogb[:], onesT[32:33, :], ogsig[:, tsl], start=True, stop=True)
            og_o, sqs = [], []
            for ft in range(2):
                ot = P_g.tile([128, LT], F32, tag="oobr", name="ot")
                nc.sync.dma_start(ot[:], dram['o_sp'][ft, :, tsl])
                oo = P_g.tile([128, LT], F32, tag="oogbr", name="oo")
                V.tensor_tensor(oo[:], ot[:], ogb[:], ALU.mult)
                og_o.append(oo)
                sq = P_g.tile([128, LT], F32, tag="osq", bufs=2, name="sq2")
                G.tensor_tensor(sq[:], oo[:], oo[:], ALU.mult)
                sqs.append(sq)
            psrm = P_gps.tile([128, LT], F32, tag="gps", name="psrm")
            for ft in range(2):
                MM(psrm[:], invdv[:], sqs[ft][:], start=(ft == 0), stop=(ft == 1))
            rrms = P_g.tile([128, LT], F32, tag="rrms", name="rrms")
            A.activation(rrms[:], psrm[:], AF.Abs_reciprocal_sqrt, bias=eps5[:, 0:1])
            for ft in range(2):
                onr = P_g.tile([128, LT], F32, tag="onr", bufs=3, name="onr")
                V.scalar_tensor_tensor(onr[:], og_o[ft][:], nw_col[:, ft:ft + 1],
                                       rrms[:], ALU.mult, ALU.mult)
                nc.sync.dma_start(dram['onrm_sp'][ft, :, tsl], onr[:])
        # ---- AG4 + output projection ----
        G.collective_compute("AllGather", ALU.bypass, replica_groups=GROUPS,
                             ins=[dram['onrm_sp'][:]], outs=[dram['ag4_out'][:]])
        for nt in range(NLT):
            tsl = slice(nt * LT, (nt + 1) * LT)
            rhs_t = []
            for k in range(KT):
                t = P_g.tile([128, LT], F32, tag="agr", bufs=10, name="rhs_t")
                nc.sync.dma_start(t[:], dram['ag4_out'][k // 2, k % 2, :, tsl])
                rhs_t.append(t)
            for mt in range(2):
                pso = P_gps.tile([128, LT], F32, tag="gps", name="pso")
                for k in range(KT):
                    MM(pso[:], wo_t[k][:, mt * 128:(mt + 1) * 128], rhs_t[k][:],
                       start=(k == 0), stop=(k == KT - 1))
                outt = P_g.tile([128, LT], F32, tag="outt", bufs=3, name="outt")
                V.tensor_copy(outt[:], pso[:])
                nc.sync.dma_start(dram['outT'][mt * 128:(mt + 1) * 128, tsl], outt[:])


def _build_msel():
    dv4 = 1.0 / (DV * 4)
    m1 = np.zeros((80, 16), np.float32)
    for j in range(4):
        for b in range(4):
            base = 20 * j + 5 * b
            m1[base + 0, 4 * b + 0] = dv4
            m1[base + 1, 4 * b + 1] = dv4
            m1[base + 4, 4 * b + 1] = -1.0 / (DV * DV * 4)
            m1[base + 2, 4 * b + 2] = dv4
            m1[base + 3, 4 * b + 3] = 0.25
    m2 = np.zeros((20, 4), np.float32)
    for j in range(4):
        base = 5 * j
        m2[base + 0, 0] = dv4
        m2[base + 1, 1] = dv4
        m2[base + 4, 1] = -1.0 / (DV * DV * 4)
        m2[base + 2, 2] = dv4
        m2[base + 3, 3] = 0.25
    return m1, m2


_NC_CACHE = None


def kernel(**inputs):
    global _NC_CACHE
    maps = build_host_inputs(inputs)
    m1, m2 = _build_msel()
    for m in maps:
        m['msel'] = m1
        m['msel2'] = m2
    if _NC_CACHE is None:
        _NC_CACHE = build_program()
    res = run_bass_kernel_spmd(_NC_CACHE, maps, list(range(8))).results
    out = np.empty((B, L, D), np.float32)
    for b in range(B):
        blocks = [res[4 * b + h]['outT'] for h in range(H)]   # (256, L) each
        out[b] = np.concatenate(blocks, axis=0).T
    return out



# revision 17
# speedup vs baseline: 2.0832x; 2.0832x over previous
"""DeltaNet fused kernel for 8 trn2 NeuronCores (bf16 rewrite).

Sharding: core c handles (b = c//4, h = c%4); heads tensor-parallel in
each 4-core group with small AllGather/AllReduce for cross-head stats.

All matmuls run bf16 (fp32 matmul is a 2-pass instruction at half rate
= ~4x slower).  PSUM stays f32; the delta state S is kept in f32 with a
bf16 mirror for MM inputs; stat rows stay f32.  Token-major operands in
the delta phase come from xbar DMA-transpose loads of the bf16 spills
instead of PE transposes.  Phase 3 runs in two token halves so its
collective chain overlaps the tail of the delta scan.
"""
import math
import os
import numpy as np
import ml_dtypes

import concourse.bass as bass
import concourse.bacc as bacc
import concourse.mybir as mybir
from concourse import tile
from concourse.bass_utils import run_bass_kernel_spmd

F32 = mybir.dt.float32
BF16 = mybir.dt.bfloat16
AF = mybir.ActivationFunctionType
ALU = mybir.AluOpType
BF_NP = ml_dtypes.bfloat16

# ---------------- static problem config ----------------
B, L, D, H = 2, 4096, 1024, 4
DK = DV = D // H          # 256
C = 128                   # delta chunk
NCHUNK = L // C           # 32
LT = 512                  # projection/streaming token tile
NLT = L // LT             # 8
KT = D // 128             # 8 contraction tiles
LH = L // 2               # phase-3 half
GROUPS = [[0, 1, 2, 3], [4, 5, 6, 7]]
EPS_RMS = 1e-5

# fir_long tap split (64 taps, tap 63 = identity):
PE_TAPS = [63] + list(range(47))   # 48 taps on PE (identity first -> f32 PSUM)
N_PE = len(PE_TAPS)
DVE_TAPS = list(range(47, 63))     # 16 taps on DVE (gpsimd lacks TensorScalarPtr)

NM = 10                   # M-tiles of 128 in the fused projection
MCOLS = NM * 128          # q 0:256 | k 256:512 | v 512:768 | g1 768:1024
                          # og1 1024:1152 | scal 1152:1160

# f32 stat-row layout in `rows` (40, L)
R_QS, R_KS = 0, 1          # raw sumsq for q/k l2norm
R_SUM, R_SQ, R_AB, R_L2, R_M2 = 8, 12, 16, 20, 24   # +g: g in {ls:0, ll:1, vd:2, do:3}
R_ORAW = 32                # o-branch raw: sum, sq, ab, l2, m2
G_LS, G_LL, G_VD, G_DO = 0, 1, 2, 3


def _np_f32(x):
    return np.ascontiguousarray(x, dtype=np.float32)


def _bf(x):
    return np.ascontiguousarray(np.asarray(x, dtype=np.float32).astype(BF_NP))


def build_host_inputs(inputs):
    """Returns per-core input maps (list of 8 dicts)."""
    hs = _np_f32(inputs['hidden_states'])
    Wq, Wk, Wv = (_np_f32(inputs[k]) for k in ('Wq', 'Wk', 'Wv'))
    Wb, Wres = _np_f32(inputs['Wb']), _np_f32(inputs['Wres'])
    Wg1, Wog1 = _np_f32(inputs['Wg1']), _np_f32(inputs['Wog1'])
    Wg2, Wog2 = _np_f32(inputs['Wg2']), _np_f32(inputs['Wog2'])
    bg2 = _np_f32(inputs['bg2'])
    wq_c, wk_c, wv_c = (_np_f32(inputs[k]) for k in ('wq_conv', 'wk_conv', 'wv_conv'))
    fir_long = _np_f32(inputs['fir_long']).reshape(D, 64)
    fir_short = _np_f32(inputs['fir_short']).reshape(D, 5)
    Wo = _np_f32(inputs['Wo'])
    logit_temp = float(np.asarray(inputs['logit_temp'])[0])
    conv_res_logit = _np_f32(inputs['conv_res_logit'])
    norm_w = _np_f32(inputs['norm_w'])
    bg1 = _np_f32(inputs['bg1']); bog1 = _np_f32(inputs['bog1'])
    bres = _np_f32(inputs['bres'])
    bog2 = float(np.asarray(inputs['bog2'])[0])

    hsT = [np.ascontiguousarray(hs[b].T) for b in range(B)]  # (D, L)

    idx = np.arange(C)
    su = (idx[:, None] < idx[None, :]).astype(np.float32)    # strict upper
    slm = (idx[:, None] > idx[None, :]).astype(np.float32)   # strict lower
    uim = (idx[:, None] <= idx[None, :]).astype(np.float32)  # upper incl
    ident = np.eye(C, dtype=np.float32)
    ident4 = np.eye(4, dtype=np.float32)

    sel44 = np.zeros((4, 4 * 128), np.float32)
    for j in range(4):
        sel44[j, j * 128:(j + 1) * 128] = 1.0
    selrs = np.zeros((4, 4), np.float32); selrs[3, 0] = 1.0
    sel8 = np.zeros((8, 4 * 128), np.float32)
    for j in range(4):
        sel8[4 + j, j * 128:(j + 1) * 128] = 1.0
    # colsel: place a 128-part reduction into row r of a 32-row psum block
    colsel = np.zeros((128, 32 * 32), np.float32)
    for r in range(32):
        colsel[:, 32 * r + r] = 1.0
    # stsel: place stc col c into row c of a 20-row psum block (identity use)

    maps = []
    for core in range(8):
        b, h = core // 4, core % 4
        sl = slice(h * 256, (h + 1) * 256)
        og_sl = slice(h * 128, (h + 1) * 128)
        wcat = np.zeros((D, MCOLS), np.float32)
        wcat[:, 0:256] = Wq[:, sl]
        wcat[:, 256:512] = Wk[:, sl]
        wcat[:, 512:768] = Wv[:, sl]
        wcat[:, 768:1024] = Wg1[:D, sl]
        wcat[:, 1024:1152] = Wog1[:D, og_sl]
        wcat[:, 1152] = Wb[:, h]
        wcat[:, 1153] = Wres[:, h]
        wq_h, wk_h, wv_h = wq_c[sl], wk_c[sl], wv_c[sl]
        fl_h, fs_h = fir_long[sl], fir_short[sl]
        # PE diag taps as one wide tile: block i = (tap PE_TAPS[i//2], ft i%2)
        diag_flat = np.zeros((128, N_PE * 2 * 128), np.float32)
        for ti, tap in enumerate(PE_TAPS):
            for ft in range(2):
                blk = (ti * 2 + ft) * 128
                w = fl_h[ft * 128:(ft + 1) * 128, tap]
                diag_flat[np.arange(128), blk + np.arange(128)] = w
        wg1_st = np.ascontiguousarray(Wg1[D:D + 16, sl])        # (16, 256)
        wog1_st = np.ascontiguousarray(Wog1[D:D + 8, og_sl])    # (8, 128)
        wg2_h = np.concatenate([Wg2[sl][:128], Wg2[sl][128:]], axis=1)  # (128, 8)
        wog2_h = np.ascontiguousarray(Wog2[og_sl, :])           # (128, 1)
        wo_cols = np.ascontiguousarray(Wo[:, sl])               # (1024, 256)
        bg1_h = np.ascontiguousarray(bg1[sl])
        bog1_h = np.ascontiguousarray(bog1[og_sl])
        inv_temp = 1.0 / math.log1p(math.exp(logit_temp))
        static_h = 1.0 / (1.0 + math.exp(-conv_res_logit[h]))
        maps.append({
            'hsT': _bf(hsT[b]),
            'wcat': _bf(wcat),
            'diag_flat': _bf(diag_flat),
            'convw': _np_f32(np.concatenate(
                [wq_h, wk_h, wv_h, fl_h, fs_h], axis=1)),       # (256, 81)
            'su': _bf(su), 'slm': _bf(slm), 'uim': _bf(uim),
            'ident': _bf(ident), 'ident4': _bf(ident4),
            'wg1_st': _bf(wg1_st), 'wog1_st': _bf(wog1_st),
            'wg2_h': _bf(wg2_h), 'wog2_h': _bf(wog2_h),
            'wo_cols': _bf(wo_cols),
            'bg1_h': bg1_h.reshape(1, -1), 'bog1_h': bog1_h.reshape(1, -1),
            'bg2': _np_f32((bg2 * inv_temp).reshape(1, 4)),
            'invt4': np.full((4, 1), inv_temp, np.float32),
            'bias2': np.array([[0.0], [bres[h]]], np.float32),
            'consts': np.array([inv_temp, static_h, bres[h], bog2,
                                norm_w[0]], np.float32).reshape(1, 5),
            'norm_w': norm_w.reshape(1, DV),
            'sres_col': np.array([[1.0], [static_h]], np.float32),
            'sel44': _bf(sel44), 'selrs': _bf(selrs), 'sel8': _bf(sel8),
            'colsel': _bf(colsel),
        })
    return maps


def _build_msel():
    """ag1 rows (20/core): [sum x4][sq x4][ab x4][l2 x4][m2 x4],
    group order within each block: (ls, ll, vd, do).
    stats16 cols: 4*b + (mean, var, am, l2), b in ref order (ls, ll, do, vd)."""
    dv4 = 1.0 / (DV * 4)
    g_of_b = [G_LS, G_LL, G_DO, G_VD]  # ref branch b -> our group index
    m1 = np.zeros((80, 16), np.float32)
    for j in range(4):
        for b in range(4):
            g = g_of_b[b]
            base = 20 * j
            m1[base + 0 + g, 4 * b + 0] = dv4
            m1[base + 4 + g, 4 * b + 1] = dv4
            m1[base + 16 + g, 4 * b + 1] = -1.0 / (DV * DV * 4)
            m1[base + 8 + g, 4 * b + 2] = dv4
            m1[base + 12 + g, 4 * b + 3] = 0.25
    m2 = np.zeros((20, 4), np.float32)
    for j in range(4):
        base = 5 * j
        m2[base + 0, 0] = dv4
        m2[base + 1, 1] = dv4
        m2[base + 4, 1] = -1.0 / (DV * DV * 4)
        m2[base + 2, 2] = dv4
        m2[base + 3, 3] = 0.25
    return _bf(m1), _bf(m2)


# ---------------- device program ----------------
PHASES = int(os.environ.get("KERN_PHASES", "3"))
NCHUNK_DBG = int(os.environ.get("KERN_NCHUNK", str(NCHUNK)))


def build_program():
    nc = bacc.Bacc("TRN2", target_bir_lowering=False, debug=True)
    dram = {}
    def din(name, shape, dt=BF16):
        dram[name] = nc.dram_tensor(name, list(shape), dt, kind="ExternalInput")
        return dram[name]
    def dout(name, shape, dt=F32):
        dram[name] = nc.dram_tensor(name, list(shape), dt, kind="ExternalOutput")
        return dram[name]
    def dint(name, shape, dt=BF16):
        dram[name] = nc.dram_tensor(name, list(shape), dt)
        return dram[name]

    din('hsT', (D, L))
    din('wcat', (D, MCOLS))
    din('diag_flat', (128, N_PE * 2 * 128))
    din('convw', (256, 81), F32)
    din('su', (C, C)); din('slm', (C, C)); din('uim', (C, C))
    din('ident', (C, C)); din('ident4', (4, 4))
    din('wg1_st', (16, 256)); din('wog1_st', (8, 128))
    din('wg2_h', (128, 8)); din('wog2_h', (128, 1))
    din('wo_cols', (D, 256))
    din('bg1_h', (1, 256), F32); din('bog1_h', (1, 128), F32)
    din('bg2', (1, 4), F32)
    din('invt4', (4, 1), F32)
    din('bias2', (2, 1), F32)
    din('consts', (1, 5), F32)
    din('sres_col', (2, 1), F32)
    din('norm_w', (1, DV), F32)
    din('msel', (80, 16)); din('msel2', (20, 4))
    din('sel44', (4, 512)); din('selrs', (4, 4)); din('sel8', (8, 512))
    din('colsel', (128, 1024))

    dout('outT', (256, L))

    for nm in ('v_sp', 'q_sp', 'k_sp', 'o_sp', 'll_sp', 'ls_sp', 'gh_sp'):
        dint(nm, (2, 128, L))
    dint('ogh_sp', (128, L))
    dint('do_sp', (L, 256))                      # token-major delta out
    dint('onrm_sp', (2, 2, 128, LH))             # [half, ft, :, :]
    dint('ag4_out', (2, 4, 2, 128, LH))
    dint('ag1_in', (2, 20, LH), F32); dint('ag1_out', (2, 80, LH), F32)
    dint('lg_in', (2, 4, LH), F32);   dint('lg_out', (2, 4, LH), F32)
    dint('ag2_in', (2, 5, LH), F32);  dint('ag2_out', (2, 20, LH), F32)
    dint('ar3_in', (2, 1, LH), F32);  dint('ar3_out', (2, 1, LH), F32)

    with tile.TileContext(nc) as tc:
        _build(nc, tc, dram)
    nc.compile()
    return nc


def _build(nc, tc, dram):
    from contextlib import ExitStack
    ctx = ExitStack()
    with ctx:
        _build_inner(nc, tc, dram, ctx)


def _build_inner(nc, tc, dram, ctx):
    MM = nc.tensor.matmul
    V, A, G = nc.vector, nc.scalar, nc.gpsimd
    X = mybir.AxisListType.X

    # ---- persistent pools ----
    P_const = ctx.enter_context(tc.tile_pool(name="const", bufs=1))
    P_big = ctx.enter_context(tc.tile_pool(name="big", bufs=1))

    def cdma(shape, dt, src, tag):
        t = P_const.tile(list(shape), dt, tag=tag)
        nc.sync.dma_start(t[:], src)
        return t

    su_sb = cdma((C, C), BF16, dram['su'][:], "su")
    slm_sb = cdma((C, C), BF16, dram['slm'][:], "slm")
    uim_sb = cdma((C, C), BF16, dram['uim'][:], "uim")
    ident_sb = cdma((C, C), BF16, dram['ident'][:], "ident")
    ident4 = cdma((4, 4), BF16, dram['ident4'][:], "ident4")
    cw = [cdma((128, 81), F32, dram['convw'][ft * 128:(ft + 1) * 128, :], f"cw{ft}")
          for ft in range(2)]
    diag_sb = cdma((128, N_PE * 2 * 128), BF16, dram['diag_flat'][:], "diag")
    wg1_st = cdma((16, 256), BF16, dram['wg1_st'][:], "wg1st")
    wog1_st = cdma((8, 128), BF16, dram['wog1_st'][:], "wog1st")
    wg2 = cdma((128, 8), BF16, dram['wg2_h'][:], "wg2")
    wog2 = cdma((128, 1), BF16, dram['wog2_h'][:], "wog2")
    msel = cdma((80, 16), BF16, dram['msel'][:], "msel")
    msel2 = cdma((20, 4), BF16, dram['msel2'][:], "msel2")
    sel44 = cdma((4, 512), BF16, dram['sel44'][:], "sel44")
    selrs = cdma((4, 4), BF16, dram['selrs'][:], "selrs")
    sel8 = P_const.tile([40, 512], BF16, tag="sel8")
    nc.sync.dma_start(sel8[32:40, :], dram['sel8'][:])
    colsel = cdma((128, 1024), BF16, dram['colsel'][:], "colsel")
    consts = cdma((1, 5), F32, dram['consts'][:], "consts")
    sres_col = cdma((2, 1), F32, dram['sres_col'][:], "srescol")
    bias2 = cdma((2, 1), F32, dram['bias2'][:], "bias2")
    invt4 = cdma((4, 1), F32, dram['invt4'][:], "invt4")
    bog1c = cdma((128, 1), F32,
                 dram['bog1_h'][:].rearrange("o (f p) -> (o p) f", p=128), "bog1c")
    bg1c = cdma((128, 2), F32,
                dram['bg1_h'][:].rearrange("o (f p) -> (o p) f", p=128), "bg1c")
    nw_col = cdma((128, 2), F32,
                  dram['norm_w'][:].rearrange("o (f p) -> (o p) f", p=128), "nwcol")
    bg2c = cdma((4, 1), F32, dram['bg2'][:].rearrange("o f -> f o"), "bg2c")
    wo_t = [cdma((128, 256), BF16, dram['wo_cols'][k * 128:(k + 1) * 128, :], f"wo{k}")
            for k in range(KT)]

    ones_col = P_const.tile([128, 1], BF16, tag="ones_col")
    G.memset(ones_col[:], 1.0)
    ones4 = P_const.tile([36, 1], BF16, tag="ones4")
    G.memset(ones4[:], 1.0)
    ones14 = P_const.tile([1, 4], BF16, tag="ones14")
    G.memset(ones14[:], 1.0)
    ones1c = P_const.tile([1, 128], BF16, tag="ones1c")
    G.memset(ones1c[:], 1.0)
    o4row = P_const.tile([1, 4], BF16, tag="o4row")
    G.memset(o4row[:], 0.02)
    invdv = P_const.tile([128, 128], BF16, tag="invdv")
    G.memset(invdv[:], 1.0 / DV)
    eps6 = P_const.tile([128, 1], F32, tag="eps6")
    G.memset(eps6[:], 1e-6)
    eps5 = P_const.tile([128, 1], F32, tag="eps5")
    G.memset(eps5[:], EPS_RMS)

    rows = P_big.tile([40, L], F32, tag="rows")
    # rowsB: [0:4] rq,rk,beta,ress | [32:40] m0..3,p0..3 | [64:72] scal rows
    rowsB = P_big.tile([72, L], BF16, tag="rowsB")
    stats16 = P_big.tile([16, L], BF16, tag="stats16")
    S_t = [P_big.tile([128, DV], F32, tag=f"S{ft}", name=f"S{ft}")
           for ft in range(2)]
    S16 = [P_big.tile([128, DV], BF16, tag=f"S16_{ft}", name=f"S16_{ft}")
           for ft in range(2)]

    CW_Q, CW_K, CW_V, CW_FL, CW_FS = 0, 4, 8, 12, 76

    # =============== PHASE 1: projections + convs ========
    with tc.tile_pool(name="wcat", bufs=1) as P_w, \
         tc.tile_pool(name="work", bufs=2) as P_work, \
         tc.tile_pool(name="ps1", bufs=3, space="PSUM") as P_ps, \
         tc.tile_pool(name="ps1c", bufs=2, space="PSUM") as P_psc, \
         tc.tile_pool(name="ps1r", bufs=2, space="PSUM") as P_psr:

        wct = []
        for k in range(KT):
            t = P_w.tile([128, MCOLS], BF16, tag=f"wcat{k}")
            nc.sync.dma_start(t[:], dram['wcat'][k * 128:(k + 1) * 128, :])
            wct.append(t)
        raws = {}
        for nm in ("q", "k", "v"):
            for ft in range(2):
                t = P_w.tile([128, 3 + LT], BF16, tag=f"raw_{nm}{ft}")
                G.memset(t[:, 0:3], 0.0)
                raws[(nm, ft)] = t
        vroll = []
        for ft in range(2):
            t = P_w.tile([128, 63 + LT], BF16, tag=f"vroll{ft}")
            G.memset(t[:, 0:63], 0.0)
            vroll.append(t)

        for lt in range(NLT):
            tsl = slice(lt * LT, (lt + 1) * LT)
            hst = []
            for k in range(KT):
                t = P_work.tile([128, LT], BF16, tag="hst", bufs=8)
                nc.sync.dma_start(t[:], dram['hsT'][k * 128:(k + 1) * 128, tsl])
                hst.append(t)
            for mt in range(NM):
                ps = P_ps.tile([128, LT], F32, tag="projps")
                for k in range(KT):
                    MM(ps[:], wct[k][:, mt * 128:(mt + 1) * 128], hst[k][:],
                       start=(k == 0), stop=(k == KT - 1))
                if mt < 6:
                    nm, ft = ("q", "k", "v")[mt // 2], mt % 2
                    buf = raws[(nm, ft)]
                    if lt > 0:
                        V.tensor_copy(buf[:, 0:3], buf[:, LT:LT + 3])
                    V.tensor_copy(buf[:, 3:3 + LT], ps[:])
                elif mt < 8:
                    t = P_work.tile([128, LT], BF16, tag="ghsb")
                    V.tensor_copy(t[:], ps[:])
                    nc.sync.dma_start(dram['gh_sp'][mt - 6, :, tsl], t[:])
                elif mt == 8:
                    t = P_work.tile([128, LT], BF16, tag="ghsb")
                    V.tensor_copy(t[:], ps[:])
                    nc.sync.dma_start(dram['ogh_sp'][:, tsl], t[:])
                else:
                    A.activation(rowsB[64:72, tsl], ps[0:8, :], AF.Copy)
            # q/k/v 4-tap conv + SiLU (identity tap already last)
            qk_tiles = {}
            for im, nm in enumerate(("q", "k", "v")):
                wof = (CW_Q, CW_K, CW_V)[im]
                for ft in range(2):
                    buf = raws[(nm, ft)]
                    acc = P_work.tile([128, LT], BF16, tag="cacc", bufs=3)
                    V.tensor_scalar(acc[:], buf[:, 0:LT], cw[ft][:, wof:wof + 1],
                                    None, ALU.mult)
                    for t_ in range(1, 4):
                        V.scalar_tensor_tensor(
                            acc[:], buf[:, t_:t_ + LT],
                            cw[ft][:, wof + t_:wof + t_ + 1],
                            acc[:], ALU.mult, ALU.add)
                    if nm == "v":
                        A.activation(vroll[ft][:, 63:63 + LT], acc[:], AF.Silu)
                        nc.sync.dma_start(dram['v_sp'][ft, :, tsl],
                                          vroll[ft][:, 63:63 + LT])
                    else:
                        t = P_work.tile([128, LT], BF16, tag="qkin", bufs=4)
                        A.activation(t[:], acc[:], AF.Silu)
                        nc.sync.dma_start(dram[f'{nm}_sp'][ft, :, tsl], t[:])
                        qk_tiles[(nm, ft)] = t
            # stat-row production into 32-row psum via colsel
            prod_ps = P_psr.tile([32, LT], F32, tag="strow")
            def prod(ridx, x_ap, first=False, last=False):
                MM(prod_ps[:], colsel[:, 32 * ridx:32 * (ridx + 1)], x_ap,
                   start=first, stop=last)
            for ridx, nm in ((R_QS, "q"), (R_KS, "k")):
                for ft in range(2):
                    x = qk_tiles[(nm, ft)]
                    sq = P_work.tile([128, LT], BF16, tag="sqsc", bufs=2)
                    G.tensor_tensor(sq[:], x[:], x[:], ALU.mult)
                    prod(ridx, sq[:], first=(ridx == R_QS and ft == 0))
            # fir_long: 45 taps on PE (incl identity), 13 DVE, 6 GPSIMD
            ll_t, ls_t = [None, None], [None, None]
            for ft in range(2):
                psll = P_psc.tile([128, LT], F32, tag="llps")
                for ti, tap in enumerate(PE_TAPS):
                    blk = (ti * 2 + ft) * 128
                    MM(psll[:], diag_sb[:, blk:blk + 128],
                       vroll[ft][:, tap:tap + LT],
                       start=(ti == 0), stop=(ti == N_PE - 1))
                t0 = DVE_TAPS[0]
                ch = P_work.tile([128, LT], BF16, tag="llch")
                V.tensor_scalar(ch[:], vroll[ft][:, t0:t0 + LT],
                                cw[ft][:, CW_FL + t0:CW_FL + t0 + 1], None,
                                ALU.mult)
                for t_ in DVE_TAPS[1:]:
                    V.scalar_tensor_tensor(ch[:], vroll[ft][:, t_:t_ + LT],
                                           cw[ft][:, CW_FL + t_:CW_FL + t_ + 1],
                                           ch[:], ALU.mult, ALU.add)
                ll = P_work.tile([128, LT], BF16, tag=f"ll{ft}")
                V.tensor_tensor(ll[:], ch[:], psll[:], ALU.add)
                nc.sync.dma_start(dram['ll_sp'][ft, :, tsl], ll[:])
                ll_t[ft] = ll
                # fir_short: taps 0..3 then identity tap 4 last
                lsb = P_work.tile([128, LT], BF16, tag=f"ls{ft}")
                V.tensor_scalar(lsb[:], vroll[ft][:, 59:59 + LT],
                                cw[ft][:, CW_FS:CW_FS + 1], None, ALU.mult)
                for t_ in range(1, 4):
                    V.scalar_tensor_tensor(lsb[:], vroll[ft][:, 59 + t_:59 + t_ + LT],
                                           cw[ft][:, CW_FS + t_:CW_FS + t_ + 1],
                                           lsb[:], ALU.mult, ALU.add)
                V.scalar_tensor_tensor(lsb[:], vroll[ft][:, 63:63 + LT],
                                       cw[ft][:, CW_FS + 4:CW_FS + 5],
                                       lsb[:], ALU.mult, ALU.add)
                nc.sync.dma_start(dram['ls_sp'][ft, :, tsl], lsb[:])
                ls_t[ft] = lsb
            # branch raw stat rows (sum, sumsq, abs) for ls, ll, vd
            for g, srcs in ((G_LS, ls_t), (G_LL, ll_t), (G_VD, None)):
                for ft in range(2):
                    x = vroll[ft][:, 63:63 + LT] if srcs is None else srcs[ft][:]
                    sq = P_work.tile([128, LT], BF16, tag="sqsc", bufs=2)
                    G.tensor_tensor(sq[:], x, x, ALU.mult)
                    ab = P_work.tile([128, LT], BF16, tag="absc", bufs=2)
                    A.activation(ab[:], x, AF.Abs)
                    last = (g == G_VD and ft == 1)
                    prod(R_SUM + g, x)
                    prod(R_SQ + g, sq[:])
                    prod(R_AB + g, ab[:], last=last)
            V.tensor_copy(rows[0:32, tsl], prod_ps[:])
            if lt < NLT - 1:
                for ft in range(2):
                    V.tensor_copy(vroll[ft][:, 0:63], vroll[ft][:, LT:LT + 63])

        # ---- row fixups ----
        # rq/rk = rsqrt(sumsq + 1e-6), bf16 rows4
        A.activation(rowsB[0:2, :], rows[0:2, :], AF.Abs_reciprocal_sqrt,
                     bias=eps6[0:2, 0:1])
        for hf in range(2):
            hsl = slice(hf * LH, (hf + 1) * LH)
            bsc = P_work.tile([2, LH], BF16, tag="bsc")
            A.activation(bsc[:], rowsB[64:66, hsl], AF.Sigmoid, bias=bias2[:, 0:1])
            V.tensor_scalar(bsc[:], bsc[:], sres_col[:, 0:1], None, ALU.mult)
            nc.sync.dma_start(rowsB[2:4, hsl], bsc[:])

    # =============== PHASE 2: delta rule (chunk C=128) ==================
    if PHASES < 2:
        _zero_out(nc, tc, dram, P_big)
        return
    with tc.tile_pool(name="dl", bufs=2) as P_dl, \
         tc.tile_pool(name="dn", bufs=3, space="PSUM") as P_dn, \
         tc.tile_pool(name="dw", bufs=3, space="PSUM") as P_dw:
        for ft in range(2):
            G.memset(S_t[ft][:], 0.0)
            G.memset(S16[ft][:], 0.0)

        def bcast(ridx, csl):
            ps = P_dn.tile([128, C], F32, tag="dn")
            MM(ps[:], sel44[:, ridx * 128:(ridx + 1) * 128], rowsB[0:4, csl],
               start=True, stop=True)
            return ps

        for cc in range(NCHUNK_DBG):
            csl = slice(cc * C, (cc + 1) * C)
            qc, kc = [], []
            for ft in range(2):
                t = P_dl.tile([128, C], BF16, tag=f"qc{ft}", bufs=3)
                nc.sync.dma_start(t[:], dram['q_sp'][ft, :, csl]); qc.append(t)
                t = P_dl.tile([128, C], BF16, tag=f"kc{ft}", bufs=3)
                nc.sync.dma_start(t[:], dram['k_sp'][ft, :, csl]); kc.append(t)
            kc_tok = P_dl.tile([128, DV], BF16, tag="kctok", bufs=3)
            vc_tok = P_dl.tile([128, DV], BF16, tag="vctok", bufs=3)
            for ft in range(2):
                nc.sync.dma_start_transpose(
                    kc_tok[:, ft * C:(ft + 1) * C], dram['k_sp'][ft, :, csl])
                nc.sync.dma_start_transpose(
                    vc_tok[:, ft * C:(ft + 1) * C], dram['v_sp'][ft, :, csl])
            ps12 = P_dn.tile([128, 4], F32, tag="ps12", bufs=1)
            MM(ps12[:], rowsB[0:4, csl], ident4[:], start=True, stop=True)
            rk_col, bcol = ps12[:, 1:2], ps12[:, 2:3]
            psb_rq = bcast(0, csl)
            psb_rk = bcast(1, csl)
            psb_beta = bcast(2, csl)
            qn, kn = [], []
            for ft in range(2):
                t = P_dl.tile([128, C], BF16, tag=f"qn{ft}")
                V.tensor_tensor(t[:], qc[ft][:], psb_rq[:], ALU.mult); qn.append(t)
                t = P_dl.tile([128, C], BF16, tag=f"kn{ft}")
                V.tensor_tensor(t[:], kc[ft][:], psb_rk[:], ALU.mult); kn.append(t)
            kn_tok = P_dl.tile([128, DV], BF16, tag="kntok")
            V.tensor_scalar(kn_tok[:], kc_tok[:], rk_col, None, ALU.mult)
            # A = kn^T kn (symmetric); B = -A*su*beta_col ; Bt = -A*slm*beta_row
            psA2 = P_dn.tile([128, C], F32, tag="dn")
            for ft in range(2):
                MM(psA2[:], kn[ft][:], kn[ft][:], start=(ft == 0), stop=(ft == 1))
            B_t = P_dl.tile([128, C], BF16, tag="B0")
            V.scalar_tensor_tensor(B_t[:], psA2[:], -1.0, su_sb[:],
                                   ALU.mult, ALU.mult)
            V.tensor_tensor(B_t[:], B_t[:], psb_beta[:], ALU.mult)
            Bt_t = P_dl.tile([128, C], BF16, tag="B1")
            V.scalar_tensor_tensor(Bt_t[:], psA2[:], -1.0, slm_sb[:],
                                   ALU.mult, ALU.mult)
            V.tensor_scalar(Bt_t[:], Bt_t[:], bcol, None, ALU.mult)
            # P = (I - B)^-1 via repeated squaring with transposed twin
            P_m = P_dl.tile([C, C], BF16, tag="Pm0")
            V.tensor_tensor(P_m[:], ident_sb[:], B_t[:], ALU.add)
            cur, curT = B_t, Bt_t
            for lvl in range(1, 7):
                if lvl < 6:
                    ps1 = P_dn.tile([C, C], F32, tag="dn")
                    MM(ps1[:], curT[:], cur[:], start=True, stop=True)
                    nxt = P_dl.tile([C, C], BF16, tag=f"nx{lvl % 2}")
                    V.tensor_copy(nxt[:], ps1[:])
                else:
                    nxt = None
                ps2 = P_dn.tile([C, C], F32, tag="dn")
                MM(ps2[:], cur[:], curT[:], start=True, stop=True)
                nxtT = P_dl.tile([C, C], BF16, tag=f"nxT{lvl % 2}")
                A.activation(nxtT[:], ps2[:], AF.Copy)
                ps3 = P_dn.tile([C, C], F32, tag="dn")
                MM(ps3[:], nxtT[:], P_m[:], start=True, stop=True)
                Pn = P_dl.tile([C, C], BF16, tag=f"Pm{1 + (lvl % 2)}")
                V.tensor_tensor(Pn[:], P_m[:], ps3[:], ALU.add)
                P_m, cur, curT = Pn, nxt, nxtT
            P_mb = P_dl.tile([C, C], BF16, tag="Pmb")
            V.tensor_scalar(P_mb[:], P_m[:], bcol, None, ALU.mult)
            # u, w^T, attn
            psu = P_dw.tile([128, DV], F32, tag="dw")
            MM(psu[:], P_mb[:], vc_tok[:], start=True, stop=True)
            wT = []
            for ft in range(2):
                psw = P_dn.tile([128, C], F32, tag="dn")
                MM(psw[:], kn_tok[:, ft * C:(ft + 1) * C], P_mb[:],
                   start=True, stop=True)
                t = P_dl.tile([128, C], BF16, tag=f"wT{ft}")
                if ft == 0:
                    V.tensor_copy(t[:], psw[:])
                else:
                    A.activation(t[:], psw[:], AF.Copy)
                wT.append(t)
            pswS = P_dw.tile([128, DV], F32, tag="dw")
            for ft in range(2):
                MM(pswS[:], wT[ft][:], S16[ft][:], start=(ft == 0), stop=(ft == 1))
            u16 = P_dl.tile([128, DV], BF16, tag="u16")
            A.activation(u16[:], psu[:], AF.Copy)
            ui2 = P_dl.tile([128, DV], BF16, tag="ui2")
            V.tensor_tensor(ui2[:], u16[:], pswS[:], ALU.subtract)
            psA = P_dn.tile([128, C], F32, tag="dn")
            for ft in range(2):
                MM(psA[:], kn[ft][:], qn[ft][:], start=(ft == 0), stop=(ft == 1))
            attnT = P_dl.tile([128, C], BF16, tag="attnT")
            V.tensor_tensor(attnT[:], psA[:], uim_sb[:], ALU.mult)
            psO = P_dw.tile([128, DV], F32, tag="dw")
            MM(psO[:], qn[0][:], S16[0][:], start=True, stop=False)
            MM(psO[:], qn[1][:], S16[1][:], start=False, stop=False)
            MM(psO[:], attnT[:], ui2[:], start=False, stop=True)
            # do + raw stats (sum, sumsq, abs over dv) -> rows 11/15/19
            stc = P_dl.tile([128, 32], F32, tag="stc")
            G.memset(stc[:], 0.0)
            do_tok = P_dl.tile([128, DV], BF16, tag="dotok")
            A.activation(do_tok[:], psO[:], AF.Copy,
                         accum_out=stc[:, R_SUM + G_DO:R_SUM + G_DO + 1])
            sqs = P_dl.tile([128, DV], BF16, tag="dosq")
            A.activation(sqs[:], psO[:], AF.Square,
                         accum_out=stc[:, R_SQ + G_DO:R_SQ + G_DO + 1])
            V.tensor_reduce(stc[:, R_AB + G_DO:R_AB + G_DO + 1], do_tok[:],
                            X, ALU.add, apply_absolute_value=True)
            stc16 = P_dl.tile([128, 32], BF16, tag="stc16")
            V.tensor_copy(stc16[:], stc[:])
            psst = P_dn.tile([32, C], F32, tag="dnst", bufs=1)
            MM(psst[:], stc16[:], ident_sb[:], start=True, stop=True)
            V.tensor_tensor(rows[0:32, csl], rows[0:32, csl], psst[:], ALU.add)
            nc.sync.dma_start(dram['do_sp'][csl, :], do_tok[:])
            # state update
            for ft in range(2):
                ps = P_dw.tile([128, DV], F32, tag="dw")
                MM(ps[:], kn_tok[:, ft * C:(ft + 1) * C], ui2[:],
                   start=True, stop=True)
                V.tensor_tensor(S_t[ft][:], S_t[ft][:], ps[:], ALU.add)
                if ft == 0:
                    A.activation(S16[ft][:], S_t[ft][:], AF.Copy)
                else:
                    G.tensor_copy(S16[ft][:], S_t[ft][:])

    # =============== PHASE 3: gates + combination + output ==============
    if PHASES < 3:
        _zero_out(nc, tc, dram, P_big)
        return

    with tc.tile_pool(name="g", bufs=2) as P_g, \
         tc.tile_pool(name="gb", bufs=4, space="PSUM") as P_gb, \
         tc.tile_pool(name="gps", bufs=2, space="PSUM") as P_gps, \
         tc.tile_pool(name="gsm", bufs=1, space="PSUM") as P_gsm, \
         tc.tile_pool(name="gsr", bufs=1, space="PSUM") as P_gsr:

        for hf in range(2):
            hsl = slice(hf * LH, (hf + 1) * LH)
            nts = range(hf * (NLT // 2), (hf + 1) * (NLT // 2))
            # derived rows: l2 = sqrt(sq), m2 = sum^2   (do rows now final)
            scrm = P_g.tile([4, LH], F32, tag="scrm", bufs=1)
            nc.sync.dma_start(scrm[:], rows[R_SUM:R_SUM + 4, hsl])
            V.tensor_tensor(scrm[:], scrm[:], scrm[:], ALU.mult)
            nc.sync.dma_start(rows[R_M2:R_M2 + 4, hsl], scrm[:])
            scrl = P_g.tile([4, LH], F32, tag="scrl", bufs=1)
            nc.sync.dma_start(scrl[:], rows[R_SQ:R_SQ + 4, hsl])
            A.activation(scrl[:], scrl[:], AF.Sqrt)
            nc.sync.dma_start(rows[R_L2:R_L2 + 4, hsl], scrl[:])
            # ---- AG1: branch raw stat rows ----
            nc.sync.dma_start(dram['ag1_in'][hf], rows[8:28, hsl])
            G.collective_compute("AllGather", ALU.bypass, replica_groups=GROUPS,
                                 ins=[dram['ag1_in'][hf]],
                                 outs=[dram['ag1_out'][hf]])
            for nt in nts:
                tsl = slice(nt * LT, (nt + 1) * LT)
                osl = slice((nt - hf * 4) * LT, (nt - hf * 4 + 1) * LT)
                agt = P_g.tile([80, LT], BF16, tag="ag80")
                G.dma_start(agt[:], dram['ag1_out'][hf, :, osl])
                ps16 = P_gsm.tile([16, LT], F32, tag="gsm")
                MM(ps16[:], msel[:], agt[:], start=True, stop=True)
                V.tensor_copy(stats16[:, tsl], ps16[:])
            # ghid + partial logits
            for nt in nts:
                tsl = slice(nt * LT, (nt + 1) * LT)
                osl = slice((nt - hf * 4) * LT, (nt - hf * 4 + 1) * LT)
                psl = P_gsm.tile([4, LT], F32, tag="gsm")
                for mt in range(2):
                    psg = P_gps.tile([128, LT], F32, tag="gps")
                    MM(psg[:], wg1_st[:, mt * 128:(mt + 1) * 128],
                       stats16[:, tsl], start=True, stop=True)
                    ghs = P_g.tile([128, LT], BF16, tag="ghs")
                    nc.sync.dma_start(ghs[:], dram['gh_sp'][mt, :, tsl])
                    pre = P_g.tile([128, LT], BF16, tag="pre")
                    V.tensor_tensor(pre[:], ghs[:], psg[:], ALU.add)
                    gha = P_g.tile([128, LT], BF16, tag="gha")
                    A.activation(gha[:], pre[:], AF.Gelu, bias=bg1c[:, mt:mt + 1])
                    MM(psl[:], wg2[:, mt * 4:(mt + 1) * 4], gha[:],
                       start=(mt == 0), stop=(mt == 1))
                lgt = P_g.tile([4, LT], F32, tag="lgt")
                V.tensor_copy(lgt[:], psl[:])
                nc.sync.dma_start(dram['lg_in'][hf, :, osl], lgt[:])
            G.collective_compute("AllReduce", ALU.add, replica_groups=GROUPS,
                                 ins=[dram['lg_in'][hf]],
                                 outs=[dram['lg_out'][hf]])
            lgf = P_g.tile([4, LH], F32, tag="lgf", bufs=1)
            trow = P_g.tile([1, LH], BF16, tag="trow", bufs=1)
            ogsig = P_g.tile([1, LH], BF16, tag="ogsig", bufs=1)
            nc.sync.dma_start(lgf[:], dram['lg_out'][hf])
            A.activation(rowsB[32:36, hsl], lgf[:], AF.Exp,
                         bias=bg2c[:, 0:1], scale=invt4[:, 0:1])
            # softmax-with-floor renorm
            for nt in nts:
                tsl = slice(nt * LT, (nt + 1) * LT)
                osl = slice((nt - hf * 4) * LT, (nt - hf * 4 + 1) * LT)
                pst = P_gsr.tile([1, LT], F32, tag="gsr")
                MM(pst[:], ones4[32:36, :], rowsB[32:36, tsl], start=True, stop=True)
                V.tensor_copy(trow[:, osl], pst[:])
                ps02 = P_gsm.tile([4, LT], F32, tag="gsm")
                MM(ps02[:], o4row[:], trow[:, osl], start=True, stop=True)
                V.tensor_tensor(rowsB[32:36, tsl], rowsB[32:36, tsl], ps02[:],
                                ALU.max)
                pst2 = P_gsr.tile([1, LT], F32, tag="gsr")
                MM(pst2[:], ones4[32:36, :], rowsB[32:36, tsl], start=True, stop=True)
                V.tensor_copy(trow[:, osl], pst2[:])
            A.activation(trow[:], trow[:], AF.Ln)
            A.activation(trow[:], trow[:], AF.Exp, scale=-1.0)
            for nt in nts:
                tsl = slice(nt * LT, (nt + 1) * LT)
                osl = slice((nt - hf * 4) * LT, (nt - hf * 4 + 1) * LT)
                ps4 = P_gsm.tile([4, LT], F32, tag="gsm")
                MM(ps4[:], ones14[:], trow[:, osl], start=True, stop=True)
                psct = P_g.tile([4, LT], BF16, tag="psct")
                V.tensor_tensor(psct[:], rowsB[32:36, tsl], ps4[:], ALU.mult)
                psrs = P_gsm.tile([4, LT], F32, tag="gsm")
                MM(psrs[:], selrs[:], rowsB[0:4, tsl], start=True, stop=True)
                pfin = P_g.tile([4, LT], BF16, tag="pfin")
                V.tensor_tensor(pfin[:], psct[:], psrs[:], ALU.add)
                nc.sync.dma_start(rowsB[36:40, tsl], pfin[:])
            # ---- combination (feat-major) + o raw stats ----
            for nt in nts:
                tsl = slice(nt * LT, (nt + 1) * LT)
                pb = []
                for j in range(4):
                    ps = P_gb.tile([128, LT], F32, tag="gb")
                    MM(ps[:], sel8[32:40, j * 128:(j + 1) * 128],
                       rowsB[32:40, tsl], start=True, stop=True)
                    sb = P_g.tile([128, LT], BF16, tag="pbs", bufs=4)
                    V.tensor_copy(sb[:], ps[:])
                    pb.append(sb)
                otiles = []
                for ft in range(2):
                    bls = P_g.tile([128, LT], BF16, tag="bls")
                    nc.sync.dma_start(bls[:], dram['ls_sp'][ft, :, tsl])
                    bll = P_g.tile([128, LT], BF16, tag="bll")
                    nc.sync.dma_start(bll[:], dram['ll_sp'][ft, :, tsl])
                    bdo = P_g.tile([128, LT], BF16, tag="bdo")
                    nc.sync.dma_start_transpose(
                        bdo[:], dram['do_sp'][tsl, ft * 128:(ft + 1) * 128])
                    bvd = P_g.tile([128, LT], BF16, tag="bvd")
                    nc.sync.dma_start(bvd[:], dram['v_sp'][ft, :, tsl])
                    o_t = P_g.tile([128, LT], BF16, tag="obr")
                    V.tensor_tensor(o_t[:], bls[:], pb[0][:], ALU.mult)
                    for br_t, j in ((bll, 1), (bdo, 2), (bvd, 3)):
                        tmp = P_g.tile([128, LT], BF16, tag="ctmp", bufs=3)
                        G.tensor_tensor(tmp[:], br_t[:], pb[j][:], ALU.mult)
                        V.tensor_tensor(o_t[:], o_t[:], tmp[:], ALU.add)
                    nc.sync.dma_start(dram['o_sp'][ft, :, tsl], o_t[:])
                    otiles.append(o_t)
                trip = []
                for ft in range(2):
                    sq = P_g.tile([128, LT], BF16, tag="osq", bufs=2)
                    G.tensor_tensor(sq[:], otiles[ft][:], otiles[ft][:], ALU.mult)
                    ab = P_g.tile([128, LT], BF16, tag="oab", bufs=2)
                    A.activation(ab[:], otiles[ft][:], AF.Abs)
                    trip.append((otiles[ft][:], sq[:], ab[:]))
                for j in range(3):
                    psr = P_gsr.tile([1, LT], F32, tag="gsr")
                    for ft in range(2):
                        MM(psr[:], ones_col[:], trip[ft][j],
                           start=(ft == 0), stop=(ft == 1))
                    orsc = P_g.tile([1, LT], F32, tag="orsc", bufs=3)
                    V.tensor_copy(orsc[:], psr[:])
                    nc.sync.dma_start(rows[R_ORAW + j:R_ORAW + j + 1, tsl],
                                      orsc[:])
            scro = P_g.tile([1, LH], F32, tag="scro", bufs=1)
            nc.sync.dma_start(scro[:], rows[R_ORAW:R_ORAW + 1, hsl])
            V.tensor_tensor(scro[:], scro[:], scro[:], ALU.mult)
            nc.sync.dma_start(rows[R_ORAW + 4:R_ORAW + 5, hsl], scro[:])
            scro2 = P_g.tile([1, LH], F32, tag="scro2", bufs=1)
            nc.sync.dma_start(scro2[:], rows[R_ORAW + 1:R_ORAW + 2, hsl])
            A.activation(scro2[:], scro2[:], AF.Sqrt)
            nc.sync.dma_start(rows[R_ORAW + 3:R_ORAW + 4, hsl], scro2[:])
            # ---- AG2 (o stats) -> og hidden -> partial og logit -> AR3 ----
            nc.sync.dma_start(dram['ag2_in'][hf], rows[R_ORAW:R_ORAW + 5, hsl])
            G.collective_compute("AllGather", ALU.bypass, replica_groups=GROUPS,
                                 ins=[dram['ag2_in'][hf]],
                                 outs=[dram['ag2_out'][hf]])
            for nt in nts:
                tsl = slice(nt * LT, (nt + 1) * LT)
                osl = slice((nt - hf * 4) * LT, (nt - hf * 4 + 1) * LT)
                ag2t = P_g.tile([20, LT], BF16, tag="ag20")
                G.dma_start(ag2t[:], dram['ag2_out'][hf, :, osl])
                ps4o = P_gsm.tile([4, LT], F32, tag="gsm")
                MM(ps4o[:], msel2[:], ag2t[:], start=True, stop=True)
                og8 = P_g.tile([8, LT], BF16, tag="og8")
                V.tensor_copy(og8[0:4, :], ps4o[:])
                nc.sync.dma_start(og8[4:8, :], stats16[12:16, tsl])
                psg = P_gps.tile([128, LT], F32, tag="gps")
                MM(psg[:], wog1_st[:], og8[:], start=True, stop=True)
                oghs = P_g.tile([128, LT], BF16, tag="ghs")
                nc.sync.dma_start(oghs[:], dram['ogh_sp'][:, tsl])
                pre2 = P_g.tile([128, LT], BF16, tag="pre")
                V.tensor_tensor(pre2[:], oghs[:], psg[:], ALU.add)
                ogha = P_g.tile([128, LT], BF16, tag="gha")
                A.activation(ogha[:], pre2[:], AF.Gelu, bias=bog1c[:, 0:1])
                psol = P_gsr.tile([1, LT], F32, tag="gsr")
                MM(psol[:], wog2[:], ogha[:], start=True, stop=True)
                ogp = P_g.tile([1, LT], F32, tag="ogp")
                V.tensor_copy(ogp[:], psol[:])
                nc.sync.dma_start(dram['ar3_in'][hf, :, osl], ogp[:])
            G.collective_compute("AllReduce", ALU.add, replica_groups=GROUPS,
                                 ins=[dram['ar3_in'][hf]],
                                 outs=[dram['ar3_out'][hf]])
            G.dma_start(ogsig[:], dram['ar3_out'][hf])
            A.activation(ogsig[:], ogsig[:], AF.Sigmoid,
                         bias=consts[0:1, 3:4])
            # ---- og apply + rmsnorm -> onrm spill ----
            for nt in nts:
                tsl = slice(nt * LT, (nt + 1) * LT)
                osl = slice((nt - hf * 4) * LT, (nt - hf * 4 + 1) * LT)
                ogb = P_gb.tile([128, LT], F32, tag="gb")
                MM(ogb[:], ones1c[:], ogsig[:, osl], start=True, stop=True)
                og_o, sqs2 = [], []
                for ft in range(2):
                    ot = P_g.tile([128, LT], BF16, tag="oobr")
                    nc.sync.dma_start(ot[:], dram['o_sp'][ft, :, tsl])
                    oo = P_g.tile([128, LT], BF16, tag="oogbr")
                    V.tensor_tensor(oo[:], ot[:], ogb[:], ALU.mult)
                    og_o.append(oo)
                    sq = P_g.tile([128, LT], BF16, tag="osq", bufs=2)
                    G.tensor_tensor(sq[:], oo[:], oo[:], ALU.mult)
                    sqs2.append(sq)
                psrm = P_gps.tile([128, LT], F32, tag="gps")
                for ft in range(2):
                    MM(psrm[:], invdv[:], sqs2[ft][:],
                       start=(ft == 0), stop=(ft == 1))
                rrms = P_g.tile([128, LT], BF16, tag="rrms")
                A.activation(rrms[:], psrm[:], AF.Abs_reciprocal_sqrt,
                             bias=eps5[:, 0:1])
                for ft in range(2):
                    onr = P_g.tile([128, LT], BF16, tag="onr", bufs=3)
                    V.scalar_tensor_tensor(onr[:], og_o[ft][:],
                                           nw_col[:, ft:ft + 1], rrms[:],
                                           ALU.mult, ALU.mult)
                    nc.sync.dma_start(dram['onrm_sp'][hf, ft, :, osl], onr[:])
            # ---- AG4 + output projection for this half ----
            G.collective_compute("AllGather", ALU.bypass, replica_groups=GROUPS,
                                 ins=[dram['onrm_sp'][hf]],
                                 outs=[dram['ag4_out'][hf]])
            for nt in nts:
                tsl = slice(nt * LT, (nt + 1) * LT)
                osl = slice((nt - hf * 4) * LT, (nt - hf * 4 + 1) * LT)
                rhs_t = []
                for k in range(KT):
                    t = P_g.tile([128, LT], BF16, tag="agr", bufs=10)
                    nc.sync.dma_start(t[:], dram['ag4_out'][hf, k // 2, k % 2, :, osl])
                    rhs_t.append(t)
                for mt in range(2):
                    pso = P_gps.tile([128, LT], F32, tag="gps")
                    for k in range(KT):
                        MM(pso[:], wo_t[k][:, mt * 128:(mt + 1) * 128], rhs_t[k][:],
                           start=(k == 0), stop=(k == KT - 1))
                    outt = P_g.tile([128, LT], F32, tag="outt", bufs=3)
                    V.tensor_copy(outt[:], pso[:])
                    nc.sync.dma_start(dram['outT'][mt * 128:(mt + 1) * 128, tsl],
                                      outt[:])


def _zero_out(nc, tc, dram, P_big):
    t0 = P_big.tile([128, LT], F32, tag="dummy")
    nc.gpsimd.memset(t0[:], 0.0)
    for mt in range(2):
        for nt in range(NLT):
            nc.sync.dma_start(
                dram['outT'][mt * 128:(mt + 1) * 128,
                             nt * LT:(nt + 1) * LT], t0[:])


_NC_CACHE = None


def kernel(**inputs):
    global _NC_CACHE
    maps = build_host_inputs(inputs)
    m1, m2 = _build_msel()
    for m in maps:
        m['msel'] = m1
        m['msel2'] = m2
    if _NC_CACHE is None:
        _NC_CACHE = build_program()
    res = run_bass_kernel_spmd(_NC_CACHE, maps, list(range(8))).results
    out = np.empty((B, L, D), np.float32)
    for b in range(B):
        blocks = [res[4 * b + h]['outT'] for h in range(H)]   # (256, L) each
        out[b] = np.concatenate(blocks, axis=0).T
    return out
